# revision 20
# baseline (speedup 1.0000x reference)
"""GCN3D segmentation network (gnn_message_passing) on 8 Trainium2 cores.

Sharding: data-parallel over batch (4 samples) x vertex-halves (2 per sample)
= 8 cores. Core c handles sample c//2, vertex half c%2 (1024 of 2048 rows).
BN statistics are all-reduced across all 8 cores; per-sample full tables
(feature maps used as gather sources / matmul inputs) are all-gathered within
each core pair. Neighbor-feature gathers run as indirect DMAs against
row-major HBM tables. KNN top-k uses the DVE max8/match_replace/max_index
instructions on a (2 q.v - |v|^2) score matrix (argmax-equivalent to argmin
of distance). Theta (direction kernels) run as K=3 bf16 matmuls with the
1/||d|| normalization folded into the ReLU eviction scale.

kernel(**inputs) takes the full unsharded reference inputs and returns the
full (feat^T, fuse^T) tuple, matching reference.reference().
"""
import sys

if "/opt/trn_rl_repo" not in sys.path:
    sys.path.insert(0, "/opt/trn_rl_repo")

import numpy as np
import ml_dtypes

import concourse.bass as bass
import concourse.bacc as bacc
import concourse.mybir as mybir
import concourse.tile as tile
from concourse.bass_utils import run_bass_kernel_spmd

F32 = mybir.dt.float32
F32R = mybir.dt.float32r
BF16 = mybir.dt.bfloat16
U32 = mybir.dt.uint32
AX = mybir.AxisListType.X
AF = mybir.ActivationFunctionType
OP = mybir.AluOpType

S = 7
NEI = 10
B = 4
V = 2048
VL = V // 2          # local rows per core
V1 = V // 4          # 512
V1L = V1 // 2        # 256
V2 = V // 16         # 128
V2L = V2 // 2        # 64
EPS = 1e-5
P = 128
PAIRS = [[0, 1], [2, 3], [4, 5], [6, 7]]
ALL8 = [list(range(8))]
NEG = -1.0e30

# conv dims: (in_c, out_c, sup=S*out_c)
OC1, SUP1 = 128, S * 128      # w1: (128, 1024)
OC2, SUP2 = 256, S * 256      # w2: (128, 2048)
OC3, SUP3 = 256, S * 256      # w3: (256, 2048)
OC4, SUP4 = 512, S * 512      # w4: (256, 4096)

_CACHE = {}
DEBUG = False


def _nchunks(n, c=512):
    out = []
    s = 0
    while s < n:
        out.append((s, min(c, n - s)))
        s += c
    return out


def _build():
    nc = bacc.Bacc("TRN2", target_bir_lowering=False, debug=False, num_devices=8)

    def inp(name, shape, dtype=F32):
        return nc.declare_dram_parameter(name, list(shape), dtype, isOutput=False)

    # ---- per-core inputs ----
    verts = inp("verts", (V, 3))            # sample vertices (gather table)
    vertsT = inp("vertsT", (3, V))
    qvT = inp("qvT", (3, VL))               # local half, feature-major
    qverts = inp("qverts", (VL, 3))         # local half rows
    qv1 = inp("qv1", (V1L, 3))              # local v1 rows
    qv1T = inp("qv1T", (3, V1L))
    rgbT = inp("rgbT", (32, VL))
    hmask = inp("hmask", (1, 1))            # h as f32 (0.0 or 1.0)
    identin = inp("identin", (P, P))
    w1 = inp("w1", (128, 128 + SUP1))
    w2 = inp("w2", (128, 256 + SUP2))
    w3 = inp("w3", (256, 256 + SUP3))
    w4 = inp("w4", (256, 512 + SUP4))
    b1r = inp("b1r", (1, 128 + SUP1))
    b2r = inp("b2r", (1, 256 + SUP2))
    b3r = inp("b3r", (1, 256 + SUP3))
    b4r = inp("b4r", (1, 512 + SUP4))
    rgw = inp("rgw", (32, 64))
    rgbbr = inp("rgbbr", (1, 64))
    sdn0 = inp("sdn0", (3, S * 64), F32R)
    sdn1 = inp("sdn1", (3, SUP1), F32R)
    sdn2 = inp("sdn2", (3, SUP2), F32R)
    sdn3 = inp("sdn3", (3, SUP3), F32R)
    sdn4 = inp("sdn4", (3, SUP4), F32R)
    bngr = inp("bngr", (1, 64 + 128 + 256 + 256))  # gammas: rgb,bn1,bn2,bn3
    bnbr = inp("bnbr", (1, 64 + 128 + 256 + 256))  # betas

    out_feat = nc.declare_dram_parameter("out_feat", [1280, VL], F32, isOutput=True)
    out_fuse = nc.declare_dram_parameter("out_fuse", [1792, VL], F32, isOutput=True)
    if DEBUG:
        dbg_nidx = nc.declare_dram_parameter("dbg_nidx", [P, 128], U32, isOutput=True)
        dbg_rinv = nc.declare_dram_parameter("dbg_rinv", [P, 80], F32, isOutput=True)
        dbg_fm0 = nc.declare_dram_parameter("dbg_fm0", [P, VL], F32, isOutput=True)
        dbg_sc0 = nc.declare_dram_parameter("dbg_sc0", [P, V], F32, isOutput=True)
        dbg_st0 = nc.declare_dram_parameter("dbg_st0", [1, 128], F32, isOutput=True)

    # ---- internal DRAM tables ----
    f1sup = nc.dram_tensor("f1sup", [V, SUP1], F32)
    f2sup = nc.dram_tensor("f2sup", [V1, SUP2], F32)
    f3sup = nc.dram_tensor("f3sup", [V1, SUP3], F32)
    f4sup = nc.dram_tensor("f4sup", [V2, SUP4], F32)
    fm1_tab = nc.dram_tensor("fm1_tab", [V, 128], F32)
    fm2_tab = nc.dram_tensor("fm2_tab", [V1, 256], F32)
    fm3_tab = nc.dram_tensor("fm3_tab", [V1, 256], F32)
    fm4_tab = nc.dram_tensor("fm4_tab", [V2, 512], F32)

    with tile.TileContext(nc) as tc:
        with (
            tc.tile_pool(name="sb", bufs=1) as sb,
            tc.tile_pool(name="st", bufs=2) as st,          # streaming sbuf
            tc.tile_pool(name="ps", bufs=2, space="PSUM") as ps,
            tc.tile_pool(name="p1", bufs=1, space="PSUM") as p1,  # wide tiles
            tc.tile_pool(name="pa", bufs=1, space="PSUM") as pa,  # accumulators
            tc.tile_pool(name="dr", bufs=1, space="DRAM") as dr,
        ):
            T = lambda shape, dtype=F32, tag=None: sb.tile(
                list(shape), dtype, tag=tag, name=tag)
            TS = lambda shape, dtype=F32, tag=None: st.tile(
                list(shape), dtype, tag=tag, name=tag)
            PT = lambda shape, tag=None, dtype=F32: ps.tile(
                list(shape), dtype, tag=tag, name=tag)
            P1 = lambda shape, tag=None: p1.tile(list(shape), F32, tag=tag, name=tag)
            PA = lambda shape, tag=None: pa.tile(list(shape), F32, tag=tag, name=tag)
            DT = lambda shape, tag, dtype=F32: dr.tile(
                list(shape), dtype, tag=tag, name=tag)

            # ================= prep =================
            # vT holds 2*verts^T (host pre-scales); score = q . (2v) - |v|^2
            vT = T((3, V), tag="vT")
            nc.sync.dma_start(out=vT[:], in_=vertsT[:])
            qvTt = T((3, VL), tag="qvTt")
            nc.sync.dma_start(out=qvTt[:], in_=qvT[:])
            qv1Tt = T((3, V1L), tag="qv1Tt")
            nc.sync.dma_start(out=qv1Tt[:], in_=qv1T[:])
            rgbTt = T((32, VL), tag="rgbTt")  # dies after surface phase
            nc.sync.dma_start(out=rgbTt[:], in_=rgbT[:])

            identf = T((P, P), tag="identf")
            nc.sync.dma_start(out=identf[:], in_=identin[:])

            ones1 = T((1, P), tag="ones1")
            nc.vector.memset(ones1[:], 1.0)
            onesc = T((P, 1), tag="onesc")
            nc.vector.memset(onesc[:], 1.0)

            # -bb row: bb = sum(v^2) = 0.25*sum((2v)^2) per vertex
            vsq = TS((P, V), tag="big8")
            nc.scalar.activation(out=vsq[:3, :], in_=vT[:], func=AF.Square)
            ones3 = T((3, 1), tag="ones3")
            nc.vector.memset(ones3[:], 1.0)
            nbb_ps = PT((1, 512), tag="sm")
            nbb = T((1, V), tag="nbb")
            for c0, cn in _nchunks(V):
                nc.tensor.matmul(out=nbb_ps[:, 0:cn], lhsT=ones3[:], rhs=vsq[:3, c0:c0 + cn],
                                 start=True, stop=True)
                nc.scalar.activation(out=nbb[:, c0:c0 + cn], in_=nbb_ps[:, 0:cn],
                                     func=AF.Copy, scale=-0.25)

            # weights / biases / dirs (phase-shared slots)
            WSM = lambda: T((128, 256 + SUP2), tag="wsmall")
            BIA = lambda: T((1, 512 + SUP4), tag="biasr")
            SDN = lambda: T((3, SUP4), F32R, tag="sdnbuf")
            w1t = WSM()
            nc.sync.dma_start(out=w1t[:, 0:128 + SUP1], in_=w1[:])
            b1t = BIA()
            nc.sync.dma_start(out=b1t[:, 0:128 + SUP1], in_=b1r[:])
            rgwt = T((32, 64), tag="rgwt")
            nc.sync.dma_start(out=rgwt[:], in_=rgw[:])
            rgbbt = T((1, 64), tag="rgbbt")
            nc.sync.dma_start(out=rgbbt[:], in_=rgbbr[:])
            sdn0t = T((3, S * 64), F32R, tag="sdn0t")
            nc.sync.dma_start(out=sdn0t[:], in_=sdn0[:])
            sdn1t = SDN()
            nc.sync.dma_start(out=sdn1t[:, 0:SUP1], in_=sdn1[:])
            bngt = T((1, 704), tag="bngt")
            nc.sync.dma_start(out=bngt[:], in_=bngr[:])
            bnbt = T((1, 704), tag="bnbt")
            nc.sync.dma_start(out=bnbt[:], in_=bnbr[:])

            hmt = T((1, 1), tag="hmt")
            nc.sync.dma_start(out=hmt[:], in_=hmask[:])
            ceps24 = T((P, 1), tag="ceps24")
            nc.vector.memset(ceps24[:], 1e-24)
            cepsr = T((1, 1), tag="cepsr")
            nc.vector.memset(cepsr[:], EPS)

            # ================= KNN1 (local 1024 queries, 2048 cands) =========
            nidx_all = T((P, 8 * 16), U32, tag="nidx_all")

            def topk16(score, idx_out):
                # score: (P, W) f32 sbuf (clobbered); idx_out: (P,16) u32 AP
                v16 = TS((P, 16), tag="v16")
                nc.vector.max(v16[:, 0:8], score)
                nc.vector.max_index(idx_out[:, 0:8], v16[:, 0:8], score)
                nc.vector.match_replace(score, v16[:, 0:8], score, NEG)
                nc.vector.max(v16[:, 8:16], score)
                nc.vector.max_index(idx_out[:, 8:16], v16[:, 8:16], score)

            def score_rows(lhsT_ap, rhsT, nbb_row, W, tag):
                # returns sbuf (P, W) score tile = 2*q.v - bb
                sc = TS((P, W), tag="big8")
                for c0, cn in _nchunks(W):
                    sp = PT((P, 512), tag="sm")
                    nc.tensor.matmul(out=sp[:, 0:cn], lhsT=lhsT_ap, rhs=rhsT[:, c0:c0 + cn],
                                     start=True, stop=False)
                    nc.tensor.matmul(out=sp[:, 0:cn], lhsT=ones1[:], rhs=nbb_row[:, c0:c0 + cn],
                                     start=False, stop=True)
                    nc.scalar.copy(out=sc[:, c0:c0 + cn], in_=sp[:, 0:cn])
                return sc

            for t in range(8):
                sc = score_rows(qvTt[:, t * P:(t + 1) * P], vT[:], nbb[:], V, "sc1")
                if DEBUG and t == 0:
                    nc.sync.dma_start(out=dbg_sc0[:], in_=sc[:])
                topk16(sc[:], nidx_all[:, t * 16:(t + 1) * 16])
            if DEBUG:
                nc.sync.dma_start(out=dbg_nidx[:], in_=nidx_all[:])

            # ================= ndn1 (shared by conv_surface & conv1) =========
            # ndn slot buffers: streaming (3, 10*128) bf16 per tile, rebuilt per
            # conv phase (coords re-gathered; rinv persists)
            _ndn_cur = {}

            def NDNB(key):
                if key not in _ndn_cur:
                    _ndn_cur[key] = T((3, 10 * P), F32R, tag="ndnb_" + key)
                return _ndn_cur[key]

            rinv = T((P, 8 * 10), tag="rinv")

            def build_ndn(t, nidx_tile, col0, verts_tab, qc_src, ndnT_dst, rinv_dst,
                          nslots=10, np_=P):
                # ndnT_dst: callable j -> (3, np_) fp32r AP destination
                # gather neighbor coords, raw = nbr - q, rinv = rsqrt(|raw|^2),
                # ndnT = transpose(bf16(raw))
                nbr = TS((np_, 3 * nslots), tag="nbr")
                for j in range(nslots):
                    nc.gpsimd.indirect_dma_start(
                        out=nbr[:, j * 3:(j + 1) * 3], out_offset=None,
                        in_=verts_tab[:],
                        in_offset=bass.IndirectOffsetOnAxis(
                            ap=nidx_tile[:, col0 + j:col0 + j + 1], axis=0))
                qc = TS((np_, 3), tag="qc")
                nc.sync.dma_start(out=qc[:], in_=qc_src)
                raw = TS((np_, 3 * nslots), tag="raw")
                nc.vector.tensor_tensor(
                    out=raw[:].rearrange("p (n c) -> p n c", n=nslots),
                    in0=nbr[:].rearrange("p (n c) -> p n c", n=nslots),
                    in1=qc[:].unsqueeze(1).to_broadcast([np_, nslots, 3]),
                    op=OP.subtract)
                sq = TS((np_, 3 * nslots), tag="sqn")
                nc.vector.tensor_tensor(out=sq[:], in0=raw[:], in1=raw[:], op=OP.mult)
                ss = TS((np_, nslots), tag="ssn")
                nc.vector.tensor_reduce(out=ss[:], in_=sq[:].rearrange("p (n c) -> p n c", n=nslots),
                                        axis=AX, op=OP.add)
                sd = TS((np_, nslots), tag="sdn_")
                nc.scalar.activation(out=sd[:], in_=ss[:], func=AF.Sqrt,
                                     bias=ceps24[:np_, 0:1])
                nc.vector.reciprocal(out=rinv_dst, in_=sd[:])
                for j in range(nslots):
                    trp = PT((3, np_), tag="sm")
                    nc.tensor.transpose(out=trp[:], in_=raw[:, 3 * j:3 * j + 3],
                                        identity=identf[:np_, :np_])
                    nc.scalar.copy(out=ndnT_dst(j), in_=trp[:])

            def ndnT(t, j):
                return NDNB(f"s1_{t % 2}")[:, j * P:(j + 1) * P]

            # ================= conv_surface + rgb -> fm_0 local ==============
            fm0_t = [T((P, 128), tag=f"rowt_{t}") for t in range(8)]
            rgs1 = PA((1, 64), tag="s1acc")
            rgs2 = PA((1, 64), tag="s2acc")
            ag0i = DT((VL, 128), tag="ag0i")
            for t in range(8):
                build_ndn(t, nidx_all[:, t * 16:(t + 1) * 16], 1, verts,
                          qverts[t * P:(t + 1) * P, :],
                          lambda j, t=t: ndnT(t, j),
                          rinv[:, t * 10:(t + 1) * 10])
                smax = st.tile([P, 448], F32, tag="smax", name="smax", bufs=1)
                for j in range(10):
                    tp = PT((P, 448), tag="sm")
                    nc.tensor.matmul(out=tp[:],
                                     lhsT=ndnT(t, j),
                                     rhs=sdn0t[:], start=True, stop=True)
                    if j == 0:
                        nc.scalar.activation(out=smax[:], in_=tp[:], func=AF.Relu,
                                             scale=rinv[:, t * 10 + j:t * 10 + j + 1])
                    else:
                        thn = TS((P, 448), tag="big8")
                        nc.scalar.activation(out=thn[:], in_=tp[:], func=AF.Relu,
                                             scale=rinv[:, t * 10 + j:t * 10 + j + 1])
                        nc.vector.tensor_tensor(out=smax[:], in0=smax[:], in1=thn[:],
                                                op=OP.max)
                srf = fm0_t[t][:, 0:64]
                nc.vector.tensor_add(out=srf, in0=smax[:, 0:64], in1=smax[:, 64:128])
                for s in range(2, 7):
                    nc.vector.tensor_add(out=srf, in0=srf, in1=smax[:, s * 64:(s + 1) * 64])
                # rgb: relu(rgbT.T @ rgw + b)
                rp = PT((P, 64), tag="sm")
                nc.tensor.matmul(out=rp[:], lhsT=rgbTt[:, t * P:(t + 1) * P], rhs=rgwt[:],
                                 start=True, stop=False)
                nc.tensor.matmul(out=rp[:], lhsT=ones1[:], rhs=rgbbt[:], start=False, stop=True)
                nc.scalar.activation(out=fm0_t[t][:, 64:128], in_=rp[:], func=AF.Relu)
                # rgb BN stats (pre-BN, post-relu)
                sqr = TS((P, 64), tag="sqr")
                nc.scalar.activation(out=sqr[:], in_=fm0_t[t][:, 64:128], func=AF.Square)
                nc.tensor.matmul(out=rgs1[:], lhsT=onesc[:], rhs=fm0_t[t][:, 64:128],
                                 start=(t == 0), stop=(t == 7))
                nc.tensor.matmul(out=rgs2[:], lhsT=onesc[:], rhs=sqr[:],
                                 start=(t == 0), stop=(t == 7))
                nc.sync.dma_start(out=ag0i[t * P:(t + 1) * P, :], in_=fm0_t[t][:])

            # stats -> bounce
            st0 = T((1, 128), tag="stx")
            nc.scalar.copy(out=st0[:, 0:64], in_=rgs1[:])
            nc.scalar.copy(out=st0[:, 64:128], in_=rgs2[:])
            ar0i = DT((1, 128), tag="ar0i")
            ar0o = DT((1, 128), tag="ar0o")
            nc.sync.dma_start(out=ar0i[:], in_=st0[:])
            nc.gpsimd.collective_compute("AllReduce", OP.add, replica_groups=ALL8,
                                         ins=[ar0i[:]], outs=[ar0o[:]])
            ag0o = DT((V, 128), tag="ag0o")
            nc.gpsimd.collective_compute("AllGather", OP.bypass, replica_groups=PAIRS,
                                         ins=[ag0i[:]], outs=[ag0o[:]])

            # ---- BN scale/bias rows from allreduced stats ----
            def bn_rows(stats_sb, Csz, n_total, g_row, b_row, tag):
                # returns (scale_row, bias_row) each (1, Csz)
                tA = st.tile([1, 256], F32, tag="bnrA", name="bnrA", bufs=1)
                tB = st.tile([1, 256], F32, tag="bnrB", name="bnrB", bufs=1)
                tC = st.tile([1, 256], F32, tag="bnrC", name="bnrC", bufs=1)
                mean, ex2, tmp = tA[:1, 0:Csz], tB[:1, 0:Csz], tC[:1, 0:Csz]
                nc.vector.tensor_scalar_mul(out=mean, in0=stats_sb[:, 0:Csz],
                                            scalar1=1.0 / n_total)
                nc.vector.tensor_scalar_mul(out=ex2, in0=stats_sb[:, Csz:2 * Csz],
                                            scalar1=1.0 / n_total)
                nc.vector.tensor_tensor(out=tmp, in0=mean, in1=mean, op=OP.mult)
                nc.vector.tensor_tensor(out=ex2, in0=ex2, in1=tmp, op=OP.subtract)
                nc.scalar.activation(out=tmp, in_=ex2, func=AF.Sqrt, bias=cepsr[:, 0:1])
                nc.vector.reciprocal(out=ex2, in_=tmp)
                scale = T((1, Csz), tag="bnr_s")
                nc.vector.tensor_tensor(out=scale[:], in0=ex2, in1=g_row, op=OP.mult)
                nc.vector.tensor_tensor(out=tmp, in0=mean, in1=scale[:], op=OP.mult)
                bias = T((1, Csz), tag="bnr_b")
                nc.vector.tensor_tensor(out=bias[:], in0=b_row, in1=tmp, op=OP.subtract)
                return scale, bias

            def bcast_rows(scale, bias, Csz, tag):
                # (1,C) -> (128,C) via PE ones outer product
                bs = T((P, Csz), tag="bnbs")
                bb = T((P, Csz), tag="bnbb")
                for row, dst in ((scale, bs), (bias, bb)):
                    bp = PT((P, Csz), tag="sm")
                    nc.tensor.matmul(out=bp[:], lhsT=ones1[:], rhs=row[:], start=True, stop=True)
                    nc.scalar.copy(out=dst[:], in_=bp[:])
                return bs, bb

            st0g = T((1, 128), tag="stxg")
            nc.sync.dma_start(out=st0g[:], in_=ar0o[:])
            sc0, bi0 = bn_rows(st0g, 64, B * V, bngt[:, 0:64], bnbt[:, 0:64], "bn0")
            bs0, bb0 = bcast_rows(sc0, bi0, 64, "bn0")

            # apply rgb BN locally; build fm0T_loc; write outputs rows 0:128
            fm0T_loc = T((P, VL), tag="fmTloc")
            for t in range(8):
                nc.vector.tensor_tensor(out=fm0_t[t][:, 64:128], in0=fm0_t[t][:, 64:128],
                                        in1=bs0[:, 0:64], op=OP.mult)
                nc.vector.tensor_tensor(out=fm0_t[t][:, 64:128], in0=fm0_t[t][:, 64:128],
                                        in1=bb0[:, 0:64], op=OP.add)
                tp = PT((P, P), tag="sm")
                nc.tensor.transpose(out=tp[:], in_=fm0_t[t][:], identity=identf[:])
                nc.scalar.copy(out=fm0T_loc[:, t * P:(t + 1) * P], in_=tp[:])
            nc.sync.dma_start(out=out_feat[0:128, :], in_=fm0T_loc[:])
            nc.sync.dma_start(out=out_fuse[0:128, :], in_=fm0T_loc[:])
            if DEBUG:
                nc.sync.dma_start(out=dbg_fm0[:], in_=fm0T_loc[:])
                nc.sync.dma_start(out=dbg_rinv[:], in_=rinv[:])
                nc.sync.dma_start(out=dbg_st0[:], in_=st0g[:])

            # full fm0T from allgather
            fm0T_full = T((P, V), tag="fmTfullA")
            for m in range(16):
                ft = TS((P, 128), tag="agt")
                nc.sync.dma_start(out=ft[:], in_=ag0o[m * P:(m + 1) * P, :])
                nc.vector.tensor_tensor(out=ft[:, 64:128], in0=ft[:, 64:128],
                                        in1=bs0[:, 0:64], op=OP.mult)
                nc.vector.tensor_tensor(out=ft[:, 64:128], in0=ft[:, 64:128],
                                        in1=bb0[:, 0:64], op=OP.add)
                tp = PT((P, P), tag="sm")
                nc.tensor.transpose(out=tp[:], in_=ft[:], identity=identf[:])
                nc.scalar.copy(out=fm0T_full[:, m * P:(m + 1) * P], in_=tp[:])

            # ================= f_out1 (pass A: support table; pass B: fc) ====
            for m in range(16):
                fo = P1((P, SUP1), tag="wide")
                for c0, cn in _nchunks(SUP1):
                    nc.tensor.matmul(out=fo[:, c0:c0 + cn],
                                     lhsT=fm0T_full[:, m * P:(m + 1) * P],
                                     rhs=w1t[:, 128 + c0:128 + c0 + cn],
                                     start=True, stop=False)
                    nc.tensor.matmul(out=fo[:, c0:c0 + cn], lhsT=ones1[:],
                                     rhs=b1t[:, 128 + c0:128 + c0 + cn],
                                     start=False, stop=True)
                fos = TS((P, SUP1), tag="big8")
                nc.scalar.copy(out=fos[:], in_=fo[:])
                nc.sync.dma_start(out=f1sup[m * P:(m + 1) * P, :], in_=fos[:])
            fc1 = T((P, VL), tag="fcbuf")
            for t in range(8):
                fp = PT((P, 128), tag="sm")
                nc.tensor.matmul(out=fp[:], lhsT=fm0T_loc[:, t * P:(t + 1) * P],
                                 rhs=w1t[:, 0:128], start=True, stop=False)
                nc.tensor.matmul(out=fp[:], lhsT=ones1[:], rhs=b1t[:, 0:128],
                                 start=False, stop=True)
                nc.scalar.copy(out=fc1[:, t * P:(t + 1) * P], in_=fp[:])

            # ================= conv1 =================
            def conv_act(t, nslots, col0, nidx_tile, sup_tab, supw, sdnt, rinv_ap,
                         ndnT_ap, oc, fc_ap, out_ap, s1ps, s2ps, first, last,
                         np_=P, eoff_half=None, acc_tag="acc1", wide=False):
                """Gather + theta + max over neighbors + sum over S + fc add.
                If eoff_half is not None: two-half gather/theta with element
                offsets (conv4); out_ap may be None (acc kept for collective)."""
                abufs = 1 if acc_tag == "acc4" else 2
                acc = st.tile([np_, supw], F32, tag=acc_tag, name=acc_tag, bufs=abufs)
                halves = [(0, supw)] if eoff_half is None else eoff_half
                for j in range(nslots):
                    for hi, (e0, en) in enumerate(halves):
                        g = TS((np_, en), tag="cvg")
                        nc.gpsimd.indirect_dma_start(
                            out=g[:], out_offset=None, in_=sup_tab[:],
                            in_offset=bass.IndirectOffsetOnAxis(
                                ap=nidx_tile[:, col0 + j:col0 + j + 1], axis=0),
                            element_offset=e0)
                        tp = P1((np_, en), tag="wide")
                        for c0, cn in _nchunks(en):
                            nc.tensor.matmul(out=tp[:, c0:c0 + cn], lhsT=ndnT_ap(j),
                                             rhs=sdnt[:, e0 + c0:e0 + c0 + cn],
                                             start=True, stop=True)
                        th = TS((np_, en), tag="big8")
                        nc.scalar.activation(out=th[:], in_=tp[:], func=AF.Relu,
                                             scale=rinv_ap(j))
                        if j == 0:
                            nc.vector.tensor_tensor(out=acc[:, e0:e0 + en], in0=th[:],
                                                    in1=g[:], op=OP.mult)
                        else:
                            nc.vector.tensor_tensor(out=th[:], in0=th[:], in1=g[:], op=OP.mult)
                            nc.vector.tensor_tensor(out=acc[:, e0:e0 + en],
                                                    in0=acc[:, e0:e0 + en], in1=th[:], op=OP.max)
                if out_ap is None:
                    return acc
                nc.vector.tensor_add(out=out_ap, in0=acc[:, 0:oc], in1=acc[:, oc:2 * oc])
                for s in range(2, S):
                    nc.vector.tensor_add(out=out_ap, in0=out_ap, in1=acc[:, s * oc:(s + 1) * oc])
                nc.vector.tensor_add(out=out_ap, in0=out_ap, in1=fc_ap)
                # stats
                sqx = TS((np_, oc), tag="cvsq")
                nc.scalar.activation(out=sqx[:], in_=out_ap, func=AF.Square)
                nc.tensor.matmul(out=s1ps[:], lhsT=onesc[:np_, :], rhs=out_ap,
                                 start=first, stop=last)
                nc.tensor.matmul(out=s2ps[:], lhsT=onesc[:np_, :], rhs=sqx[:],
                                 start=first, stop=last)
                return None

            fm1p_t = [T((P, 128), tag=f"rowt_{t}") for t in range(8)]
            s1_1 = PA((1, 128), tag="s1acc")
            s2_1 = PA((1, 128), tag="s2acc")
            ag1i = DT((VL, 128), tag="ag1i")
            for t in range(8):
                build_ndn(t, nidx_all[:, t * 16:(t + 1) * 16], 1, verts,
                          qverts[t * P:(t + 1) * P, :],
                          lambda j, t=t: ndnT(t, j),
                          rinv[:, t * 10:(t + 1) * 10])
                conv_act(t, 10, t * 16 + 1, nidx_all, f1sup, SUP1, sdn1t,
                         lambda j, t=t: rinv[:, t * 10 + j:t * 10 + j + 1],
                         lambda j, t=t: ndnT(t, j),
                         128, fc1[:, t * P:(t + 1) * P], fm1p_t[t][:],
                         s1_1, s2_1, t == 0, t == 7, acc_tag="cvacc")
                nc.sync.dma_start(out=ag1i[t * P:(t + 1) * P, :], in_=fm1p_t[t][:])

            st1 = T((1, 256), tag="stx")
            nc.scalar.copy(out=st1[:, 0:128], in_=s1_1[:])
            nc.scalar.copy(out=st1[:, 128:256], in_=s2_1[:])
            ar1i = DT((1, 256), tag="ar1i")
            ar1o = DT((1, 256), tag="ar1o")
            nc.sync.dma_start(out=ar1i[:], in_=st1[:])
            nc.gpsimd.collective_compute("AllReduce", OP.add, replica_groups=ALL8,
                                         ins=[ar1i[:]], outs=[ar1o[:]])
            ag1o = DT((V, 128), tag="ag1o")
            nc.gpsimd.collective_compute("AllGather", OP.bypass, replica_groups=PAIRS,
                                         ins=[ag1i[:]], outs=[ag1o[:]])

            st1g = T((1, 256), tag="stxg")
            nc.sync.dma_start(out=st1g[:], in_=ar1o[:])
            sc1r, bi1r = bn_rows(st1g, 128, B * V, bngt[:, 64:192], bnbt[:, 64:192], "bn1")
            bs1, bb1 = bcast_rows(sc1r, bi1r, 128, "bn1")

            fm1T_loc = T((P, VL), tag="fmTloc")
            for t in range(8):
                nc.vector.tensor_tensor(out=fm1p_t[t][:], in0=fm1p_t[t][:], in1=bs1[:], op=OP.mult)
                nc.vector.tensor_tensor(out=fm1p_t[t][:], in0=fm1p_t[t][:], in1=bb1[:], op=OP.add)
                nc.scalar.activation(out=fm1p_t[t][:], in_=fm1p_t[t][:], func=AF.Relu)
                tp = PT((P, P), tag="sm")
                nc.tensor.transpose(out=tp[:], in_=fm1p_t[t][:], identity=identf[:])
                nc.scalar.copy(out=fm1T_loc[:, t * P:(t + 1) * P], in_=tp[:])
            nc.sync.dma_start(out=out_feat[128:256, :], in_=fm1T_loc[:])
            nc.sync.dma_start(out=out_fuse[128:256, :], in_=fm1T_loc[:])

            for m in range(16):
                ft = TS((P, 128), tag="agt")
                nc.sync.dma_start(out=ft[:], in_=ag1o[m * P:(m + 1) * P, :])
                nc.vector.tensor_tensor(out=ft[:], in0=ft[:], in1=bs1[:], op=OP.mult)
                nc.vector.tensor_tensor(out=ft[:], in0=ft[:], in1=bb1[:], op=OP.add)
                nc.scalar.activation(out=ft[:], in_=ft[:], func=AF.Relu)
                nc.sync.dma_start(out=fm1_tab[m * P:(m + 1) * P, :], in_=ft[:])

            # ================= pool1 =================
            # pooled_nidx: nidx cols 1..4 at local rows 4*j
            p1n = [T((P, 4), U32, tag=f"p1n_{jt}") for jt in range(2)]
            for jt in range(2):
                for tp_ in range(4):
                    t = 4 * jt + tp_
                    nc.sync.dma_start(
                        out=p1n[jt][tp_ * 32:(tp_ + 1) * 32, 0:4],
                        in_=nidx_all[0::4, t * 16 + 1:t * 16 + 5])
            fmp1_t = [T((P, 128), tag=f"fmp1_{jt}") for jt in range(2)]
            agp1i = DT((V1L, 128), tag="agp1i")
            for jt in range(2):
                for i in range(4):
                    pg = TS((P, 128), tag="gu")
                    nc.gpsimd.indirect_dma_start(
                        out=pg[:], out_offset=None, in_=fm1_tab[:],
                        in_offset=bass.IndirectOffsetOnAxis(ap=p1n[jt][:, i:i + 1], axis=0))
                    if i == 0:
                        nc.vector.tensor_copy(out=fmp1_t[jt][:], in_=pg[:])
                    else:
                        nc.vector.tensor_tensor(out=fmp1_t[jt][:], in0=fmp1_t[jt][:],
                                                in1=pg[:], op=OP.max)
                nc.sync.dma_start(out=agp1i[jt * P:(jt + 1) * P, :], in_=fmp1_t[jt][:])
            agp1o = DT((V1, 128), tag="agp1o")
            nc.gpsimd.collective_compute("AllGather", OP.bypass, replica_groups=PAIRS,
                                         ins=[agp1i[:]], outs=[agp1o[:]])
            fmp1T_loc = T((P, V1L), tag="fmTlocB")
            for jt in range(2):
                tp = PT((P, P), tag="sm")
                nc.tensor.transpose(out=tp[:], in_=fmp1_t[jt][:], identity=identf[:])
                nc.scalar.copy(out=fmp1T_loc[:, jt * P:(jt + 1) * P], in_=tp[:])
            fmp1T_full = T((P, V1), tag="fmTfullB")
            for m in range(4):
                ft = TS((P, 128), tag="agt")
                nc.sync.dma_start(out=ft[:], in_=agp1o[m * P:(m + 1) * P, :])
                tp = PT((P, P), tag="sm")
                nc.tensor.transpose(out=tp[:], in_=ft[:], identity=identf[:])
                nc.scalar.copy(out=fmp1T_full[:, m * P:(m + 1) * P], in_=tp[:])

            # ================= KNN2 (local 256 queries over 512) ============
            nidx1 = T((P, 2 * 16), U32, tag="nidx1")
            vT2_1 = vT[:, 0::4]
            nbb1 = nbb[:, 0::4]
            for tt in range(2):
                sc = score_rows(qv1Tt[:, tt * P:(tt + 1) * P], vT2_1, nbb1, V1, "sc2")
                topk16(sc[:], nidx1[:, tt * 16:(tt + 1) * 16])
            nidx1x4 = T((P, 2 * 16), U32, tag="nidx1x4")
            nc.vector.tensor_scalar(out=nidx1x4[:], in0=nidx1[:], scalar1=2, scalar2=None,
                                    op0=OP.logical_shift_left)

            rinv2 = T((P, 2 * 10), tag="rinv2")

            def ndn2T(tt, j):
                return NDNB(f"s1_{tt % 2}")[:, j * P:(j + 1) * P]

            # ================= conv2 =================
            w2t = WSM()
            nc.sync.dma_start(out=w2t[:], in_=w2[:])
            b2t = BIA()
            nc.sync.dma_start(out=b2t[:, 0:256 + SUP2], in_=b2r[:])
            sdn2t = SDN()
            nc.sync.dma_start(out=sdn2t[:, 0:SUP2], in_=sdn2[:])

            for m in range(4):
                fo = P1((P, SUP2), tag="wide")
                for c0, cn in _nchunks(SUP2):
                    nc.tensor.matmul(out=fo[:, c0:c0 + cn],
                                     lhsT=fmp1T_full[:, m * P:(m + 1) * P],
                                     rhs=w2t[:, 256 + c0:256 + c0 + cn], start=True, stop=False)
                    nc.tensor.matmul(out=fo[:, c0:c0 + cn], lhsT=ones1[:],
                                     rhs=b2t[:, 256 + c0:256 + c0 + cn], start=False, stop=True)
                fos = TS((P, SUP2), tag="big8")
                nc.scalar.copy(out=fos[:], in_=fo[:])
                nc.sync.dma_start(out=f2sup[m * P:(m + 1) * P, :], in_=fos[:])
            fc2 = T((P, 2 * 256), tag="fcbuf")
            for tt in range(2):
                fp = PT((P, 256), tag="sm")
                nc.tensor.matmul(out=fp[:], lhsT=fmp1T_loc[:, tt * P:(tt + 1) * P],
                                 rhs=w2t[:, 0:256], start=True, stop=False)
                nc.tensor.matmul(out=fp[:], lhsT=ones1[:], rhs=b2t[:, 0:256],
                                 start=False, stop=True)
                nc.scalar.copy(out=fc2[:, tt * 256:(tt + 1) * 256], in_=fp[:])

            fm2p_t = [T((P, 256), tag=f"rowt2_{tt}") for tt in range(2)]
            s1_2 = PA((1, 256), tag="s1acc")
            s2_2 = PA((1, 256), tag="s2acc")
            ag2i = DT((V1L, 256), tag="ag2i")
            for tt in range(2):
                build_ndn(tt, nidx1x4[:, tt * 16:(tt + 1) * 16], 1, verts,
                          qv1[tt * P:(tt + 1) * P, :],
                          lambda j, tt=tt: ndn2T(tt, j),
                          rinv2[:, tt * 10:(tt + 1) * 10])
                conv_act(tt, 10, tt * 16 + 1, nidx1, f2sup, SUP2, sdn2t,
                         lambda j, tt=tt: rinv2[:, tt * 10 + j:tt * 10 + j + 1],
                         lambda j, tt=tt: ndn2T(tt, j),
                         256, fc2[:, tt * 256:(tt + 1) * 256], fm2p_t[tt][:],
                         s1_2, s2_2, tt == 0, tt == 1, acc_tag="cvacc", wide=True)
                nc.sync.dma_start(out=ag2i[tt * P:(tt + 1) * P, :], in_=fm2p_t[tt][:])

            st2 = T((1, 512), tag="stx")
            nc.scalar.copy(out=st2[:, 0:256], in_=s1_2[:])
            nc.scalar.copy(out=st2[:, 256:512], in_=s2_2[:])
            ar2i = DT((1, 512), tag="ar2i")
            ar2o = DT((1, 512), tag="ar2o")
            nc.sync.dma_start(out=ar2i[:], in_=st2[:])
            nc.gpsimd.collective_compute("AllReduce", OP.add, replica_groups=ALL8,
                                         ins=[ar2i[:]], outs=[ar2o[:]])
            ag2o = DT((V1, 256), tag="ag2o")
            nc.gpsimd.collective_compute("AllGather", OP.bypass, replica_groups=PAIRS,
                                         ins=[ag2i[:]], outs=[ag2o[:]])

            st2g = T((1, 512), tag="stxg")
            nc.sync.dma_start(out=st2g[:], in_=ar2o[:])
            sc2r, bi2r = bn_rows(st2g, 256, B * V1, bngt[:, 192:448], bnbt[:, 192:448], "bn2")
            bs2, bb2 = bcast_rows(sc2r, bi2r, 256, "bn2")

            # local fm2 (for conv3 pass B lhsT)
            fm2T_loc = [T((P, V1L), tag=("fmTlocB" if k == 0 else "fmTlocC")) for k in range(2)]
            for tt in range(2):
                nc.vector.tensor_tensor(out=fm2p_t[tt][:], in0=fm2p_t[tt][:], in1=bs2[:], op=OP.mult)
                nc.vector.tensor_tensor(out=fm2p_t[tt][:], in0=fm2p_t[tt][:], in1=bb2[:], op=OP.add)
                nc.scalar.activation(out=fm2p_t[tt][:], in_=fm2p_t[tt][:], func=AF.Relu)
                for k in range(2):
                    tp = PT((P, P), tag="sm")
                    nc.tensor.transpose(out=tp[:], in_=fm2p_t[tt][:, k * P:(k + 1) * P],
                                        identity=identf[:])
                    nc.scalar.copy(out=fm2T_loc[k][:, tt * P:(tt + 1) * P], in_=tp[:])
            fm2T_full = [T((P, V1), tag=("fmTfullA" if k == 0 else "fmTfullB")) for k in range(2)]
            for m in range(4):
                ft = TS((P, 256), tag="agt")
                nc.sync.dma_start(out=ft[:], in_=ag2o[m * P:(m + 1) * P, :])
                nc.vector.tensor_tensor(out=ft[:], in0=ft[:], in1=bs2[:], op=OP.mult)
                nc.vector.tensor_tensor(out=ft[:], in0=ft[:], in1=bb2[:], op=OP.add)
                nc.scalar.activation(out=ft[:], in_=ft[:], func=AF.Relu)
                nc.sync.dma_start(out=fm2_tab[m * P:(m + 1) * P, :], in_=ft[:])
                for k in range(2):
                    tp = PT((P, P), tag="sm")
                    nc.tensor.transpose(out=tp[:], in_=ft[:, k * P:(k + 1) * P],
                                        identity=identf[:])
                    nc.scalar.copy(out=fm2T_full[k][:, m * P:(m + 1) * P], in_=tp[:])

            # ================= conv3 =================
            b3t = BIA()
            nc.sync.dma_start(out=b3t[:, 0:256 + SUP3], in_=b3r[:])
            sdn3t = SDN()
            nc.sync.dma_start(out=sdn3t[:, 0:SUP3], in_=sdn3[:])

            for m in range(4):
                fo = P1((P, SUP3), tag="wide")
                for c0, cn in _nchunks(SUP3):
                    wa = TS((P, 512), tag="wAc")
                    nc.sync.dma_start(out=wa[:, 0:cn], in_=w3[0:128, 256 + c0:256 + c0 + cn])
                    wb = TS((P, 512), tag="wBc")
                    nc.sync.dma_start(out=wb[:, 0:cn], in_=w3[128:256, 256 + c0:256 + c0 + cn])
                    nc.tensor.matmul(out=fo[:, c0:c0 + cn],
                                     lhsT=fm2T_full[0][:, m * P:(m + 1) * P],
                                     rhs=wa[:, 0:cn], start=True, stop=False)
                    nc.tensor.matmul(out=fo[:, c0:c0 + cn],
                                     lhsT=fm2T_full[1][:, m * P:(m + 1) * P],
                                     rhs=wb[:, 0:cn], start=False, stop=False)
                    nc.tensor.matmul(out=fo[:, c0:c0 + cn], lhsT=ones1[:],
                                     rhs=b3t[:, 256 + c0:256 + c0 + cn], start=False, stop=True)
                fos = TS((P, SUP3), tag="big8")
                nc.scalar.copy(out=fos[:], in_=fo[:])
                nc.sync.dma_start(out=f3sup[m * P:(m + 1) * P, :], in_=fos[:])
            fc3 = T((P, 2 * 256), tag="fcbuf")
            for tt in range(2):
                wa = TS((P, 512), tag="wAc")
                nc.sync.dma_start(out=wa[:, 0:256], in_=w3[0:128, 0:256])
                wb = TS((P, 512), tag="wBc")
                nc.sync.dma_start(out=wb[:, 0:256], in_=w3[128:256, 0:256])
                fp = PT((P, 256), tag="sm")
                nc.tensor.matmul(out=fp[:], lhsT=fm2T_loc[0][:, tt * P:(tt + 1) * P],
                                 rhs=wa[:, 0:256], start=True, stop=False)
                nc.tensor.matmul(out=fp[:], lhsT=fm2T_loc[1][:, tt * P:(tt + 1) * P],
                                 rhs=wb[:, 0:256], start=False, stop=False)
                nc.tensor.matmul(out=fp[:], lhsT=ones1[:], rhs=b3t[:, 0:256],
                                 start=False, stop=True)
                nc.scalar.copy(out=fc3[:, tt * 256:(tt + 1) * 256], in_=fp[:])

            fm3p_t = [T((P, 256), tag=f"rowt2_{tt}") for tt in range(2)]
            s1_3 = PA((1, 256), tag="s1acc")
            s2_3 = PA((1, 256), tag="s2acc")
            ag3i = DT((V1L, 256), tag="ag3i")
            for tt in range(2):
                build_ndn(tt, nidx1x4[:, tt * 16:(tt + 1) * 16], 1, verts,
                          qv1[tt * P:(tt + 1) * P, :],
                          lambda j, tt=tt: ndn2T(tt, j),
                          rinv2[:, tt * 10:(tt + 1) * 10])
                conv_act(tt, 10, tt * 16 + 1, nidx1, f3sup, SUP3, sdn3t,
                         lambda j, tt=tt: rinv2[:, tt * 10 + j:tt * 10 + j + 1],
                         lambda j, tt=tt: ndn2T(tt, j),
                         256, fc3[:, tt * 256:(tt + 1) * 256], fm3p_t[tt][:],
                         s1_3, s2_3, tt == 0, tt == 1, acc_tag="cvacc", wide=True)
                nc.sync.dma_start(out=ag3i[tt * P:(tt + 1) * P, :], in_=fm3p_t[tt][:])

            st3 = T((1, 512), tag="stx")
            nc.scalar.copy(out=st3[:, 0:256], in_=s1_3[:])
            nc.scalar.copy(out=st3[:, 256:512], in_=s2_3[:])
            ar3i = DT((1, 512), tag="ar3i")
            ar3o = DT((1, 512), tag="ar3o")
            nc.sync.dma_start(out=ar3i[:], in_=st3[:])
            nc.gpsimd.collective_compute("AllReduce", OP.add, replica_groups=ALL8,
                                         ins=[ar3i[:]], outs=[ar3o[:]])
            ag3o = DT((V1, 256), tag="ag3o")
            nc.gpsimd.collective_compute("AllGather", OP.bypass, replica_groups=PAIRS,
                                         ins=[ag3i[:]], outs=[ag3o[:]])

            st3g = T((1, 512), tag="stxg")
            nc.sync.dma_start(out=st3g[:], in_=ar3o[:])
            sc3r, bi3r = bn_rows(st3g, 256, B * V1, bngt[:, 448:704], bnbt[:, 448:704], "bn3")
            bs3, bb3 = bcast_rows(sc3r, bi3r, 256, "bn3")

            for m in range(4):
                ft = TS((P, 256), tag="agt")
                nc.sync.dma_start(out=ft[:], in_=ag3o[m * P:(m + 1) * P, :])
                nc.vector.tensor_tensor(out=ft[:], in0=ft[:], in1=bs3[:], op=OP.mult)
                nc.vector.tensor_tensor(out=ft[:], in0=ft[:], in1=bb3[:], op=OP.add)
                nc.scalar.activation(out=ft[:], in_=ft[:], func=AF.Relu)
                nc.sync.dma_start(out=fm3_tab[m * P:(m + 1) * P, :], in_=ft[:])

            # ================= pool2 + fmp2 =================
            p2n = T((V2L, 4), U32, tag="p2n")
            nc.sync.dma_start(out=p2n[0:32, 0:4], in_=nidx1[0::4, 1:5])
            nc.sync.dma_start(out=p2n[32:64, 0:4], in_=nidx1[0::4, 17:21])
            fmp2_loc = T((V2L, 256), tag="fmp2_loc")
            for i in range(4):
                pg = TS((V2L, 256), tag="gu")
                nc.gpsimd.indirect_dma_start(
                    out=pg[:], out_offset=None, in_=fm3_tab[:],
                    in_offset=bass.IndirectOffsetOnAxis(ap=p2n[:, i:i + 1], axis=0))
                if i == 0:
                    nc.vector.tensor_copy(out=fmp2_loc[:], in_=pg[:])
                else:
                    nc.vector.tensor_tensor(out=fmp2_loc[:], in0=fmp2_loc[:], in1=pg[:], op=OP.max)
            agp2i = DT((V2L, 256), tag="agp2i")
            nc.sync.dma_start(out=agp2i[:], in_=fmp2_loc[:])
            agp2o = DT((V2, 256), tag="agp2o")
            nc.gpsimd.collective_compute("AllGather", OP.bypass, replica_groups=PAIRS,
                                         ins=[agp2i[:]], outs=[agp2o[:]])
            fmp2f = T((P, 256), tag="fmp2f")
            nc.sync.dma_start(out=fmp2f[:], in_=agp2o[:])
            fmp2T = [T((P, P), tag=f"fmp2T{k}") for k in range(2)]
            for k in range(2):
                tp = PT((P, P), tag="sm")
                nc.tensor.transpose(out=tp[:], in_=fmp2f[:, k * P:(k + 1) * P], identity=identf[:])
                nc.scalar.copy(out=fmp2T[k][:], in_=tp[:])

            # ================= KNN3 (all 128 over 128) =================
            nidx2 = T((P, 16), U32, tag="nidx2")
            qv2 = TS((P, 3), tag="qc")
            nc.sync.dma_start(out=qv2[:], in_=verts[0::16, :])
            qv2T_ps = PT((3, P), tag="sm")
            nc.tensor.transpose(out=qv2T_ps[:], in_=qv2[:], identity=identf[:])
            qv2T = T((3, P), tag="qv2T")
            nc.scalar.copy(out=qv2T[:], in_=qv2T_ps[:])
            sc = score_rows(qv2T[:], vT[:, 0::16], nbb[:, 0::16], V2, "sc3")
            topk16(sc[:], nidx2[:])
            # blend even/odd neighbor slots by h: core h takes slots {1+h,3+h,..}
            hb_ps = PT((P, 1), tag="sm")
            nc.tensor.matmul(out=hb_ps[:], lhsT=ones1[:], rhs=hmt[:], start=True, stop=True)
            hb = T((P, 1), tag="hb")
            nc.scalar.copy(out=hb[:], in_=hb_ps[:])
            ihb = T((P, 1), tag="ihb")
            nc.vector.tensor_scalar(out=ihb[:], in0=hb[:], scalar1=-1.0, scalar2=1.0,
                                    op0=OP.mult, op1=OP.add)
            n2f = T((P, 16), tag="n2f")
            nc.vector.tensor_copy(out=n2f[:], in_=nidx2[:])
            seln_f = T((P, 5), tag="seln_f")
            for j in range(5):
                a = TS((P, 1), tag="bl_a")
                nc.vector.tensor_tensor(out=a[:], in0=n2f[:, 1 + 2 * j:2 + 2 * j],
                                        in1=ihb[:], op=OP.mult)
                bsel = TS((P, 1), tag="bl_b")
                nc.vector.tensor_tensor(out=bsel[:], in0=n2f[:, 2 + 2 * j:3 + 2 * j],
                                        in1=hb[:], op=OP.mult)
                nc.vector.tensor_tensor(out=seln_f[:, j:j + 1], in0=a[:], in1=bsel[:], op=OP.add)
            seln = T((P, 5), U32, tag="seln")
            nc.vector.tensor_copy(out=seln[:], in_=seln_f[:])
            selnx16 = T((P, 5), U32, tag="selnx16")
            nc.vector.tensor_scalar(out=selnx16[:], in0=seln[:], scalar1=4, scalar2=None,
                                    op0=OP.logical_shift_left)

            rinv3 = T((P, 5), tag="rinv3")

            def ndn3T(j):
                return NDNB("s1_0")[:, j * P:(j + 1) * P]

            build_ndn(0, selnx16, 0, verts, verts[0::16, :],
                      lambda j: ndn3T(j), rinv3[:], nslots=5)

            # ================= f_out4 =================
            b4t = BIA()
            nc.sync.dma_start(out=b4t[:], in_=b4r[:])
            sdn4t = SDN()
            nc.sync.dma_start(out=sdn4t[:], in_=sdn4[:])

            fc4 = T((P, 512), tag="fcbuf")
            for ch in range(4):
                fo = P1((P, 1024), tag="wide")
                base = ch * 1024
                for c0, cn in _nchunks(1024):
                    wa = TS((P, 512), tag="wAc")
                    nc.sync.dma_start(out=wa[:, 0:cn], in_=w4[0:128, base + c0:base + c0 + cn])
                    wb = TS((P, 512), tag="wBc")
                    nc.sync.dma_start(out=wb[:, 0:cn], in_=w4[128:256, base + c0:base + c0 + cn])
                    nc.tensor.matmul(out=fo[:, c0:c0 + cn], lhsT=fmp2T[0][:],
                                     rhs=wa[:, 0:cn], start=True, stop=False)
                    nc.tensor.matmul(out=fo[:, c0:c0 + cn], lhsT=fmp2T[1][:],
                                     rhs=wb[:, 0:cn], start=False, stop=False)
                    nc.tensor.matmul(out=fo[:, c0:c0 + cn], lhsT=ones1[:],
                                     rhs=b4t[:, base + c0:base + c0 + cn], start=False, stop=True)
                fos = TS((P, 1024), tag="big8")
                nc.scalar.copy(out=fos[:], in_=fo[:])
                if ch == 0:
                    nc.vector.tensor_copy(out=fc4[:], in_=fos[:, 0:512])
                    nc.sync.dma_start(out=f4sup[:, 0:512], in_=fos[:, 512:1024])
                else:
                    nc.sync.dma_start(out=f4sup[:, base - 512:base + 512], in_=fos[:])

            # ================= conv4 (slot-split 5/5, max-allreduce) =========
            acc4 = conv_act(0, 5, 0, seln, f4sup, SUP4, sdn4t,
                            lambda j: rinv3[:, j:j + 1],
                            lambda j: ndn3T(j),
                            512, None, None, None, None, True, True,
                            eoff_half=[(0, 1792), (1792, 1792)], acc_tag="acc4", wide=True)
            ar4i = DT((P, SUP4), tag="ar4i")
            nc.sync.dma_start(out=ar4i[:], in_=acc4[:])
            ar4o = DT((P, SUP4), tag="ar4o")
            nc.gpsimd.collective_compute("AllReduce", OP.max, replica_groups=PAIRS,
                                         ins=[ar4i[:]], outs=[ar4o[:]])
            acc4f = st.tile([P, SUP4], F32, tag="acc4", name="acc4f", bufs=1)
            nc.sync.dma_start(out=acc4f[:], in_=ar4o[:])
            fm4 = T((P, 512), tag="fm4")
            nc.vector.tensor_add(out=fm4[:], in0=acc4f[:, 0:512], in1=acc4f[:, 512:1024])
            for s in range(2, S):
                nc.vector.tensor_add(out=fm4[:], in0=fm4[:], in1=acc4f[:, s * 512:(s + 1) * 512])
            nc.vector.tensor_add(out=fm4[:], in0=fm4[:], in1=fc4[:])
            nc.sync.dma_start(out=fm4_tab[:], in_=fm4[:])

            # f_global: max over the 128 v2 rows -> (512,), broadcast to fuse rows
            for k in range(4):
                tp = PT((P, P), tag="sm")
                nc.tensor.transpose(out=tp[:], in_=fm4[:, k * P:(k + 1) * P], identity=identf[:])
                fmT = TS((P, P), tag="fm4T")
                nc.scalar.copy(out=fmT[:], in_=tp[:])
                fg = TS((P, 1), tag="fg")
                nc.vector.tensor_reduce(out=fg[:], in_=fmT[:], axis=AX, op=OP.max)
                fgb = TS((P, VL), tag="big8")
                nc.vector.tensor_scalar(out=fgb[:], in0=fm1T_loc[:], scalar1=0.0,
                                        scalar2=fg[:, 0:1], op0=OP.mult, op1=OP.add)
                nc.sync.dma_start(out=out_fuse[1280 + k * P:1280 + (k + 1) * P, :], in_=fgb[:])

            # ================= near1/near2 + upsample =================
            vT2_2 = vT[:, 0::16]
            nbb2 = nbb[:, 0::16]
            for t in range(8):
                sc1 = score_rows(qvTt[:, t * P:(t + 1) * P], vT2_1, nbb1, V1, "scn1")
                v8 = TS((P, 8), tag="v8a")
                i8a = T((P, 8), U32, tag="i8a")
                nc.vector.max(v8[:], sc1[:])
                nc.vector.max_index(i8a[:], v8[:], sc1[:])
                sc2 = score_rows(qvTt[:, t * P:(t + 1) * P], vT2_2, nbb2, V2, "scn2")
                v8b = TS((P, 8), tag="v8b")
                i8b = T((P, 8), U32, tag="i8b")
                nc.vector.max(v8b[:], sc2[:])
                nc.vector.max_index(i8b[:], v8b[:], sc2[:])

                for tab, idx_t, width, r0 in (
                    (fm2_tab, i8a, 256, 256),
                    (fm3_tab, i8a, 256, 512),
                    (fm4_tab, i8b, 512, 768),
                ):
                    gu = TS((P, width), tag="gu")
                    nc.gpsimd.indirect_dma_start(
                        out=gu[:], out_offset=None, in_=tab[:],
                        in_offset=bass.IndirectOffsetOnAxis(ap=idx_t[:, 0:1], axis=0))
                    for k in range(width // P):
                        tp = PT((P, P), tag="sm")
                        nc.tensor.transpose(out=tp[:], in_=gu[:, k * P:(k + 1) * P],
                                            identity=identf[:])
                        ot = TS((P, P), tag="otile")
                        nc.scalar.copy(out=ot[:], in_=tp[:])
                        nc.sync.dma_start(
                            out=out_feat[r0 + k * P:r0 + (k + 1) * P, t * P:(t + 1) * P],
                            in_=ot[:])
                        nc.sync.dma_start(
                            out=out_fuse[r0 + k * P:r0 + (k + 1) * P, t * P:(t + 1) * P],
                            in_=ot[:])

    nc.compile()
    return nc


def _prep_in_maps(vertices, rgb_f, dir0, w1, b1, dir1, w2, b2, dir2, w3, b3, dir3,
                  w4, b4, dir4, rgb_w, rgb_b, rgb_bn_g, rgb_bn_b,
                  bn1_g, bn1_b, bn2_g, bn2_b, bn3_g, bn3_b):
    f32 = np.float32
    bf16 = ml_dtypes.bfloat16

    def norm_cols(d):
        n = np.linalg.norm(d.astype(np.float64), axis=0)
        return (d / np.maximum(n, 1e-12)).astype(f32)

    sdns = [norm_cols(d) for d in (dir0, dir1, dir2, dir3, dir4)]
    bng = np.concatenate([rgb_bn_g, bn1_g, bn2_g, bn3_g]).reshape(1, -1).astype(f32)
    bnb = np.concatenate([rgb_bn_b, bn1_b, bn2_b, bn3_b]).reshape(1, -1).astype(f32)
    ident = np.eye(P, dtype=f32)

    shared = dict(
        identin=ident,
        w1=np.ascontiguousarray(w1, f32), w2=np.ascontiguousarray(w2, f32),
        w3=np.ascontiguousarray(w3, f32), w4=np.ascontiguousarray(w4, f32),
        b1r=b1.reshape(1, -1).astype(f32), b2r=b2.reshape(1, -1).astype(f32),
        b3r=b3.reshape(1, -1).astype(f32), b4r=b4.reshape(1, -1).astype(f32),
        rgw=np.ascontiguousarray(rgb_w.T, f32), rgbbr=rgb_b.reshape(1, -1).astype(f32),
        sdn0=sdns[0], sdn1=sdns[1], sdn2=sdns[2], sdn3=sdns[3], sdn4=sdns[4],
        bngr=bng, bnbr=bnb,
    )
    in_maps = []
    for c in range(8):
        s, h = c // 2, c % 2
        vs = np.ascontiguousarray(vertices[s], f32)           # (V,3)
        vsT = np.ascontiguousarray(vs.T, f32)                 # (3,V)
        m = dict(shared)
        m["verts"] = vs
        m["vertsT"] = np.ascontiguousarray(2.0 * vsT, f32)
        m["qvT"] = np.ascontiguousarray(vsT[:, h * VL:(h + 1) * VL], f32)
        m["qverts"] = np.ascontiguousarray(vs[h * VL:(h + 1) * VL, :], f32)
        m["qv1"] = np.ascontiguousarray(vs[h * VL:(h + 1) * VL:4, :], f32)
        m["qv1T"] = np.ascontiguousarray(vs[h * VL:(h + 1) * VL:4, :].T, f32)
        m["rgbT"] = np.ascontiguousarray(rgb_f[s][:, h * VL:(h + 1) * VL], f32)
        m["hmask"] = np.array([[float(h)]], f32)
        in_maps.append(m)
    return in_maps


def kernel(**inputs):
    if "nc" not in _CACHE:
        _CACHE["nc"] = _build()
    nc = _CACHE["nc"]
    in_maps = _prep_in_maps(**inputs)
    res = run_bass_kernel_spmd(nc, in_maps, list(range(8))).results
    feat = np.stack([
        np.concatenate([res[2 * s]["out_feat"], res[2 * s + 1]["out_feat"]], axis=1)
        for s in range(B)
    ]).astype(np.float32)
    fuse = np.stack([
        np.concatenate([res[2 * s]["out_fuse"], res[2 * s + 1]["out_fuse"]], axis=1)
        for s in range(B)
    ]).astype(np.float32)
    return feat, fuse


# revision 22
# speedup vs baseline: 58.7198x; 58.7198x over previous
"""GCN3D segmentation network (gnn_message_passing) on 8 Trainium2 cores.

Sharding: data-parallel over batch (4 samples) x vertex-halves (2 per sample)
= 8 cores. Core c handles sample c//2, vertex half c%2 (1024 of 2048 rows).
BN statistics are all-reduced across all 8 cores; per-sample full tables
(feature maps used as gather sources / matmul inputs) are all-gathered within
each core pair. Neighbor-feature gathers run as indirect DMAs against
row-major HBM tables. KNN top-k uses the DVE max8/match_replace/max_index
instructions on a (2 q.v - |v|^2) score matrix (argmax-equivalent to argmin
of distance). Theta (direction kernels) run as K=3 bf16 matmuls with the
1/||d|| normalization folded into the ReLU eviction scale.

kernel(**inputs) takes the full unsharded reference inputs and returns the
full (feat^T, fuse^T) tuple, matching reference.reference().
"""
import sys

if "/opt/trn_rl_repo" not in sys.path:
    sys.path.insert(0, "/opt/trn_rl_repo")

import numpy as np
import ml_dtypes

import concourse.bass as bass
import concourse.bacc as bacc
import concourse.mybir as mybir
import concourse.tile as tile
from concourse.bass_utils import run_bass_kernel_spmd

F32 = mybir.dt.float32
F32R = mybir.dt.float32r
BF16 = mybir.dt.bfloat16
U32 = mybir.dt.uint32
AX = mybir.AxisListType.X
AF = mybir.ActivationFunctionType
OP = mybir.AluOpType

S = 7
NEI = 10
B = 4
V = 2048
VL = V // 2          # local rows per core
V1 = V // 4          # 512
V1L = V1 // 2        # 256
V2 = V // 16         # 128
V2L = V2 // 2        # 64
EPS = 1e-5
P = 128
PAIRS = [[0, 1], [2, 3], [4, 5], [6, 7]]
ALL8 = [list(range(8))]
NEG = -1.0e30

# conv dims: (in_c, out_c, sup=S*out_c)
OC1, SUP1 = 128, S * 128      # w1: (128, 1024)
OC2, SUP2 = 256, S * 256      # w2: (128, 2048)
OC3, SUP3 = 256, S * 256      # w3: (256, 2048)
OC4, SUP4 = 512, S * 512      # w4: (256, 4096)

_CACHE = {}
DEBUG = False


def _nchunks(n, c=512):
    out = []
    s = 0
    while s < n:
        out.append((s, min(c, n - s)))
        s += c
    return out


def _build():
    nc = bacc.Bacc("TRN2", target_bir_lowering=False, debug=False, num_devices=8)

    def inp(name, shape, dtype=F32):
        return nc.declare_dram_parameter(name, list(shape), dtype, isOutput=False)

    # ---- per-core inputs ----
    verts = inp("verts", (V, 3))            # sample vertices (gather table)
    vertsT = inp("vertsT", (3, V))
    qvT = inp("qvT", (3, VL))               # local half, feature-major
    qverts = inp("qverts", (VL, 3))         # local half rows
    qv1 = inp("qv1", (V1L, 3))              # local v1 rows
    qv1T = inp("qv1T", (3, V1L))
    rgbT = inp("rgbT", (32, VL))
    hmask = inp("hmask", (1, 1))            # h as f32 (0.0 or 1.0)
    identin = inp("identin", (P, P))
    w1 = inp("w1", (128, 128 + SUP1))
    w2 = inp("w2", (128, 256 + SUP2))
    w3 = inp("w3", (256, 256 + SUP3))
    w4 = inp("w4", (256, 512 + SUP4))
    b1r = inp("b1r", (1, 128 + SUP1))
    b2r = inp("b2r", (1, 256 + SUP2))
    b3r = inp("b3r", (1, 256 + SUP3))
    b4r = inp("b4r", (1, 512 + SUP4))
    rgw = inp("rgw", (32, 64))
    rgbbr = inp("rgbbr", (1, 64))
    sdn0 = inp("sdn0", (3, S * 64), F32R)
    sdn1 = inp("sdn1", (3, SUP1), F32R)
    sdn2 = inp("sdn2", (3, SUP2), F32R)
    sdn3 = inp("sdn3", (3, SUP3), F32R)
    sdn4 = inp("sdn4", (3, SUP4), F32R)
    bngr = inp("bngr", (1, 64 + 128 + 256 + 256))  # gammas: rgb,bn1,bn2,bn3
    bnbr = inp("bnbr", (1, 64 + 128 + 256 + 256))  # betas

    out_fuse = nc.declare_dram_parameter("out_fuse", [1792, VL], F32, isOutput=True)
    if DEBUG:
        dbg_nidx = nc.declare_dram_parameter("dbg_nidx", [P, 128], U32, isOutput=True)
        dbg_rinv = nc.declare_dram_parameter("dbg_rinv", [P, 80], F32, isOutput=True)
        dbg_fm0 = nc.declare_dram_parameter("dbg_fm0", [P, VL], F32, isOutput=True)
        dbg_sc0 = nc.declare_dram_parameter("dbg_sc0", [P, V], F32, isOutput=True)
        dbg_st0 = nc.declare_dram_parameter("dbg_st0", [1, 128], F32, isOutput=True)

    # ---- internal DRAM tables ----
    f1sup = nc.dram_tensor("f1sup", [V, SUP1], F32)
    f2sup = nc.dram_tensor("f2sup", [V1, SUP2], F32)
    f3sup = nc.dram_tensor("f3sup", [V1, SUP3], F32)
    f4sup = nc.dram_tensor("f4sup", [V2, SUP4], F32)
    fm1_tab = nc.dram_tensor("fm1_tab", [V, 128], F32)
    fm2_tab = nc.dram_tensor("fm2_tab", [V1, 256], F32)
    fm3_tab = nc.dram_tensor("fm3_tab", [V1, 256], F32)
    fm4_tab = nc.dram_tensor("fm4_tab", [V2, 512], F32)

    with tile.TileContext(nc) as tc:
        with (
            tc.tile_pool(name="sb", bufs=1) as sb,
            tc.tile_pool(name="st", bufs=2) as st,          # streaming sbuf
            tc.tile_pool(name="ps", bufs=2, space="PSUM") as ps,
            tc.tile_pool(name="p1", bufs=1, space="PSUM") as p1,  # wide tiles
            tc.tile_pool(name="pa", bufs=1, space="PSUM") as pa,  # accumulators
            tc.tile_pool(name="dr", bufs=1, space="DRAM") as dr,
        ):
            T = lambda shape, dtype=F32, tag=None: sb.tile(
                list(shape), dtype, tag=tag, name=tag)
            TS = lambda shape, dtype=F32, tag=None: st.tile(
                list(shape), dtype, tag=tag, name=tag)
            PT = lambda shape, tag=None, dtype=F32: ps.tile(
                list(shape), dtype, tag=tag, name=tag)
            P1 = lambda shape, tag=None: p1.tile(list(shape), F32, tag=tag, name=tag)
            PA = lambda shape, tag=None: pa.tile(list(shape), F32, tag=tag, name=tag)
            DT = lambda shape, tag, dtype=F32: dr.tile(
                list(shape), dtype, tag=tag, name=tag)

            # ================= prep =================
            # vT holds 2*verts^T (host pre-scales); score = q . (2v) - |v|^2
            vT = T((3, V), tag="vT")
            nc.sync.dma_start(out=vT[:], in_=vertsT[:])
            qvTt = T((3, VL), tag="qvTt")
            nc.sync.dma_start(out=qvTt[:], in_=qvT[:])
            qv1Tt = T((3, V1L), tag="qv1Tt")
            nc.sync.dma_start(out=qv1Tt[:], in_=qv1T[:])
            rgbTt = T((32, VL), tag="rgbTt")  # dies after surface phase
            nc.sync.dma_start(out=rgbTt[:], in_=rgbT[:])

            identf = T((P, P), tag="identf")
            nc.sync.dma_start(out=identf[:], in_=identin[:])

            ones1 = T((1, P), tag="ones1")
            nc.vector.memset(ones1[:], 1.0)
            onesc = T((P, 1), tag="onesc")
            nc.vector.memset(onesc[:], 1.0)

            # -bb row: bb = sum(v^2) = 0.25*sum((2v)^2) per vertex
            vsq = TS((P, V), tag="big8")
            nc.scalar.activation(out=vsq[:3, :], in_=vT[:], func=AF.Square)
            ones3 = T((3, 1), tag="ones3")
            nc.vector.memset(ones3[:], 1.0)
            nbb_ps = PT((1, 512), tag="sm")
            nbb = T((1, V), tag="nbb")
            for c0, cn in _nchunks(V):
                nc.tensor.matmul(out=nbb_ps[:, 0:cn], lhsT=ones3[:], rhs=vsq[:3, c0:c0 + cn],
                                 start=True, stop=True)
                nc.scalar.activation(out=nbb[:, c0:c0 + cn], in_=nbb_ps[:, 0:cn],
                                     func=AF.Copy, scale=-0.25)

            # weights / biases / dirs (phase-shared slots)
            WSM = lambda: T((128, 256 + SUP2), tag="wsmall")
            BIA = lambda: T((1, 512 + SUP4), tag="biasr")
            SDN = lambda: T((3, SUP4), F32R, tag="sdnbuf")
            w1t = WSM()
            nc.sync.dma_start(out=w1t[:, 0:128 + SUP1], in_=w1[:])
            b1t = BIA()
            nc.sync.dma_start(out=b1t[:, 0:128 + SUP1], in_=b1r[:])
            rgwt = T((32, 64), tag="rgwt")
            nc.sync.dma_start(out=rgwt[:], in_=rgw[:])
            rgbbt = T((1, 64), tag="rgbbt")
            nc.sync.dma_start(out=rgbbt[:], in_=rgbbr[:])
            sdn0t = T((3, S * 64), F32R, tag="sdn0t")
            nc.sync.dma_start(out=sdn0t[:], in_=sdn0[:])
            sdn1t = SDN()
            nc.sync.dma_start(out=sdn1t[:, 0:SUP1], in_=sdn1[:])
            bngt = T((1, 704), tag="bngt")
            nc.sync.dma_start(out=bngt[:], in_=bngr[:])
            bnbt = T((1, 704), tag="bnbt")
            nc.sync.dma_start(out=bnbt[:], in_=bnbr[:])

            hmt = T((1, 1), tag="hmt")
            nc.sync.dma_start(out=hmt[:], in_=hmask[:])
            ceps24 = T((P, 1), tag="ceps24")
            nc.vector.memset(ceps24[:], 1e-24)
            cepsr = T((1, 1), tag="cepsr")
            nc.vector.memset(cepsr[:], EPS)

            # ================= KNN1 (local 1024 queries, 2048 cands) =========
            nidx_all = T((P, 8 * 16), U32, tag="nidx_all")

            def topk16(score, idx_out):
                # score: (P, W) f32 sbuf (clobbered); idx_out: (P,16) u32 AP
                v16 = TS((P, 16), tag="v16")
                nc.vector.max(v16[:, 0:8], score)
                nc.vector.max_index(idx_out[:, 0:8], v16[:, 0:8], score)
                nc.vector.match_replace(score, v16[:, 0:8], score, NEG)
                nc.vector.max(v16[:, 8:16], score)
                nc.vector.max_index(idx_out[:, 8:16], v16[:, 8:16], score)

            def score_rows(lhsT_ap, rhsT, nbb_row, W, tag):
                # returns sbuf (P, W) score tile = 2*q.v - bb
                sc = TS((P, W), tag="big8")
                for c0, cn in _nchunks(W):
                    sp = PT((P, 512), tag="sm")
                    nc.tensor.matmul(out=sp[:, 0:cn], lhsT=lhsT_ap, rhs=rhsT[:, c0:c0 + cn],
                                     start=True, stop=False)
                    nc.tensor.matmul(out=sp[:, 0:cn], lhsT=ones1[:], rhs=nbb_row[:, c0:c0 + cn],
                                     start=False, stop=True)
                    nc.scalar.copy(out=sc[:, c0:c0 + cn], in_=sp[:, 0:cn])
                return sc

            for t in range(8):
                sc = score_rows(qvTt[:, t * P:(t + 1) * P], vT[:], nbb[:], V, "sc1")
                if DEBUG and t == 0:
                    nc.sync.dma_start(out=dbg_sc0[:], in_=sc[:])
                topk16(sc[:], nidx_all[:, t * 16:(t + 1) * 16])
            if DEBUG:
                nc.sync.dma_start(out=dbg_nidx[:], in_=nidx_all[:])

            # ================= ndn1 (shared by conv_surface & conv1) =========
            # ndn slot buffers: streaming (3, 10*128) bf16 per tile, rebuilt per
            # conv phase (coords re-gathered; rinv persists)
            _ndn_cur = {}

            def NDNB(key):
                if key not in _ndn_cur:
                    _ndn_cur[key] = T((3, 10 * P), F32R, tag="ndnb_" + key)
                return _ndn_cur[key]

            rinv = T((P, 8 * 10), tag="rinv")

            def build_ndn(t, nidx_tile, col0, verts_tab, qc_src, ndnT_dst, rinv_dst,
                          nslots=10, np_=P):
                # ndnT_dst: callable j -> (3, np_) fp32r AP destination
                # gather neighbor coords, raw = nbr - q, rinv = rsqrt(|raw|^2),
                # ndnT = transpose(bf16(raw))
                nbr = TS((np_, 3 * nslots), tag="nbr")
                for j in range(nslots):
                    nc.gpsimd.indirect_dma_start(
                        out=nbr[:, j * 3:(j + 1) * 3], out_offset=None,
                        in_=verts_tab[:],
                        in_offset=bass.IndirectOffsetOnAxis(
                            ap=nidx_tile[:, col0 + j:col0 + j + 1], axis=0))
                qc = TS((np_, 3), tag="qc")
                nc.sync.dma_start(out=qc[:], in_=qc_src)
                raw = TS((np_, 3 * nslots), tag="raw")
                nc.vector.tensor_tensor(
                    out=raw[:].rearrange("p (n c) -> p n c", n=nslots),
                    in0=nbr[:].rearrange("p (n c) -> p n c", n=nslots),
                    in1=qc[:].unsqueeze(1).to_broadcast([np_, nslots, 3]),
                    op=OP.subtract)
                sq = TS((np_, 3 * nslots), tag="sqn")
                nc.vector.tensor_tensor(out=sq[:], in0=raw[:], in1=raw[:], op=OP.mult)
                ss = TS((np_, nslots), tag="ssn")
                nc.vector.tensor_reduce(out=ss[:], in_=sq[:].rearrange("p (n c) -> p n c", n=nslots),
                                        axis=AX, op=OP.add)
                sd = TS((np_, nslots), tag="sdn_")
                nc.scalar.activation(out=sd[:], in_=ss[:], func=AF.Sqrt,
                                     bias=ceps24[:np_, 0:1])
                nc.vector.reciprocal(out=rinv_dst, in_=sd[:])
                for j in range(nslots):
                    trp = PT((3, np_), tag="sm")
                    nc.tensor.transpose(out=trp[:], in_=raw[:, 3 * j:3 * j + 3],
                                        identity=identf[:np_, :np_])
                    nc.scalar.copy(out=ndnT_dst(j), in_=trp[:])

            def ndnT(t, j):
                return NDNB(f"s1_{t % 2}")[:, j * P:(j + 1) * P]

            # ================= conv_surface + rgb -> fm_0 local ==============
            fm0_t = [T((P, 128), tag=f"rowt_{t}") for t in range(8)]
            rgs1 = PA((1, 64), tag="s1acc")
            rgs2 = PA((1, 64), tag="s2acc")
            ag0i = DT((VL, 128), tag="ag0i")
            for t in range(8):
                build_ndn(t, nidx_all[:, t * 16:(t + 1) * 16], 1, verts,
                          qverts[t * P:(t + 1) * P, :],
                          lambda j, t=t: ndnT(t, j),
                          rinv[:, t * 10:(t + 1) * 10])
                smax = st.tile([P, 448], F32, tag="smax", name="smax", bufs=1)
                for j in range(10):
                    tp = PT((P, 448), tag="sm")
                    nc.tensor.matmul(out=tp[:],
                                     lhsT=ndnT(t, j),
                                     rhs=sdn0t[:], start=True, stop=True)
                    if j == 0:
                        nc.scalar.activation(out=smax[:], in_=tp[:], func=AF.Relu,
                                             scale=rinv[:, t * 10 + j:t * 10 + j + 1])
                    else:
                        thn = TS((P, 448), tag="big8")
                        nc.scalar.activation(out=thn[:], in_=tp[:], func=AF.Relu,
                                             scale=rinv[:, t * 10 + j:t * 10 + j + 1])
                        nc.vector.tensor_tensor(out=smax[:], in0=smax[:], in1=thn[:],
                                                op=OP.max)
                srf = fm0_t[t][:, 0:64]
                nc.vector.tensor_add(out=srf, in0=smax[:, 0:64], in1=smax[:, 64:128])
                for s in range(2, 7):
                    nc.vector.tensor_add(out=srf, in0=srf, in1=smax[:, s * 64:(s + 1) * 64])
                # rgb: relu(rgbT.T @ rgw + b)
                rp = PT((P, 64), tag="sm")
                nc.tensor.matmul(out=rp[:], lhsT=rgbTt[:, t * P:(t + 1) * P], rhs=rgwt[:],
                                 start=True, stop=False)
                nc.tensor.matmul(out=rp[:], lhsT=ones1[:], rhs=rgbbt[:], start=False, stop=True)
                nc.scalar.activation(out=fm0_t[t][:, 64:128], in_=rp[:], func=AF.Relu)
                # rgb BN stats (pre-BN, post-relu)
                sqr = TS((P, 64), tag="sqr")
                nc.scalar.activation(out=sqr[:], in_=fm0_t[t][:, 64:128], func=AF.Square)
                nc.tensor.matmul(out=rgs1[:], lhsT=onesc[:], rhs=fm0_t[t][:, 64:128],
                                 start=(t == 0), stop=(t == 7))
                nc.tensor.matmul(out=rgs2[:], lhsT=onesc[:], rhs=sqr[:],
                                 start=(t == 0), stop=(t == 7))
                nc.sync.dma_start(out=ag0i[t * P:(t + 1) * P, :], in_=fm0_t[t][:])

            # stats -> bounce
            st0 = T((1, 128), tag="stx")
            nc.scalar.copy(out=st0[:, 0:64], in_=rgs1[:])
            nc.scalar.copy(out=st0[:, 64:128], in_=rgs2[:])
            ar0i = DT((1, 128), tag="ar0i")
            ar0o = DT((1, 128), tag="ar0o")
            nc.sync.dma_start(out=ar0i[:], in_=st0[:])
            nc.gpsimd.collective_compute("AllReduce", OP.add, replica_groups=ALL8,
                                         ins=[ar0i[:]], outs=[ar0o[:]])
            ag0o = DT((V, 128), tag="ag0o")
            nc.gpsimd.collective_compute("AllGather", OP.bypass, replica_groups=PAIRS,
                                         ins=[ag0i[:]], outs=[ag0o[:]])

            # ---- BN scale/bias rows from allreduced stats ----
            def bn_rows(stats_sb, Csz, n_total, g_row, b_row, tag):
                # returns (scale_row, bias_row) each (1, Csz)
                tA = st.tile([1, 256], F32, tag="bnrA", name="bnrA", bufs=1)
                tB = st.tile([1, 256], F32, tag="bnrB", name="bnrB", bufs=1)
                tC = st.tile([1, 256], F32, tag="bnrC", name="bnrC", bufs=1)
                mean, ex2, tmp = tA[:1, 0:Csz], tB[:1, 0:Csz], tC[:1, 0:Csz]
                nc.vector.tensor_scalar_mul(out=mean, in0=stats_sb[:, 0:Csz],
                                            scalar1=1.0 / n_total)
                nc.vector.tensor_scalar_mul(out=ex2, in0=stats_sb[:, Csz:2 * Csz],
                                            scalar1=1.0 / n_total)
                nc.vector.tensor_tensor(out=tmp, in0=mean, in1=mean, op=OP.mult)
                nc.vector.tensor_tensor(out=ex2, in0=ex2, in1=tmp, op=OP.subtract)
                nc.scalar.activation(out=tmp, in_=ex2, func=AF.Sqrt, bias=cepsr[:, 0:1])
                nc.vector.reciprocal(out=ex2, in_=tmp)
                scale = T((1, Csz), tag="bnr_s")
                nc.vector.tensor_tensor(out=scale[:], in0=ex2, in1=g_row, op=OP.mult)
                nc.vector.tensor_tensor(out=tmp, in0=mean, in1=scale[:], op=OP.mult)
                bias = T((1, Csz), tag="bnr_b")
                nc.vector.tensor_tensor(out=bias[:], in0=b_row, in1=tmp, op=OP.subtract)
                return scale, bias

            def bcast_rows(scale, bias, Csz, tag):
                # (1,C) -> (128,C) via PE ones outer product
                bs = T((P, Csz), tag="bnbs")
                bb = T((P, Csz), tag="bnbb")
                for row, dst in ((scale, bs), (bias, bb)):
                    bp = PT((P, Csz), tag="sm")
                    nc.tensor.matmul(out=bp[:], lhsT=ones1[:], rhs=row[:], start=True, stop=True)
                    nc.scalar.copy(out=dst[:], in_=bp[:])
                return bs, bb

            st0g = T((1, 128), tag="stxg")
            nc.sync.dma_start(out=st0g[:], in_=ar0o[:])
            sc0, bi0 = bn_rows(st0g, 64, B * V, bngt[:, 0:64], bnbt[:, 0:64], "bn0")
            bs0, bb0 = bcast_rows(sc0, bi0, 64, "bn0")

            # apply rgb BN locally; build fm0T_loc; write outputs rows 0:128
            fm0T_loc = T((P, VL), tag="fmTloc")
            for t in range(8):
                nc.vector.tensor_tensor(out=fm0_t[t][:, 64:128], in0=fm0_t[t][:, 64:128],
                                        in1=bs0[:, 0:64], op=OP.mult)
                nc.vector.tensor_tensor(out=fm0_t[t][:, 64:128], in0=fm0_t[t][:, 64:128],
                                        in1=bb0[:, 0:64], op=OP.add)
                tp = PT((P, P), tag="sm")
                nc.tensor.transpose(out=tp[:], in_=fm0_t[t][:], identity=identf[:])
                nc.scalar.copy(out=fm0T_loc[:, t * P:(t + 1) * P], in_=tp[:])
            nc.sync.dma_start(out=out_fuse[0:128, :], in_=fm0T_loc[:])
            if DEBUG:
                nc.sync.dma_start(out=dbg_fm0[:], in_=fm0T_loc[:])
                nc.sync.dma_start(out=dbg_rinv[:], in_=rinv[:])
                nc.sync.dma_start(out=dbg_st0[:], in_=st0g[:])

            # full fm0T from allgather
            fm0T_full = T((P, V), tag="fmTfullA")
            for m in range(16):
                ft = TS((P, 128), tag="agt")
                nc.sync.dma_start(out=ft[:], in_=ag0o[m * P:(m + 1) * P, :])
                nc.vector.tensor_tensor(out=ft[:, 64:128], in0=ft[:, 64:128],
                                        in1=bs0[:, 0:64], op=OP.mult)
                nc.vector.tensor_tensor(out=ft[:, 64:128], in0=ft[:, 64:128],
                                        in1=bb0[:, 0:64], op=OP.add)
                tp = PT((P, P), tag="sm")
                nc.tensor.transpose(out=tp[:], in_=ft[:], identity=identf[:])
                nc.scalar.copy(out=fm0T_full[:, m * P:(m + 1) * P], in_=tp[:])

            # ================= f_out1 (pass A: support table; pass B: fc) ====
            for m in range(16):
                fo = P1((P, SUP1), tag="wide")
                for c0, cn in _nchunks(SUP1):
                    nc.tensor.matmul(out=fo[:, c0:c0 + cn],
                                     lhsT=fm0T_full[:, m * P:(m + 1) * P],
                                     rhs=w1t[:, 128 + c0:128 + c0 + cn],
                                     start=True, stop=False)
                    nc.tensor.matmul(out=fo[:, c0:c0 + cn], lhsT=ones1[:],
                                     rhs=b1t[:, 128 + c0:128 + c0 + cn],
                                     start=False, stop=True)
                fos = TS((P, SUP1), tag="big8")
                nc.scalar.copy(out=fos[:], in_=fo[:])
                nc.sync.dma_start(out=f1sup[m * P:(m + 1) * P, :], in_=fos[:])
            fc1 = T((P, VL), tag="fcbuf")
            for t in range(8):
                fp = PT((P, 128), tag="sm")
                nc.tensor.matmul(out=fp[:], lhsT=fm0T_loc[:, t * P:(t + 1) * P],
                                 rhs=w1t[:, 0:128], start=True, stop=False)
                nc.tensor.matmul(out=fp[:], lhsT=ones1[:], rhs=b1t[:, 0:128],
                                 start=False, stop=True)
                nc.scalar.copy(out=fc1[:, t * P:(t + 1) * P], in_=fp[:])

            # ================= conv1 =================
            def conv_act(t, nslots, col0, nidx_tile, sup_tab, supw, sdnt, rinv_ap,
                         ndnT_ap, oc, fc_ap, out_ap, s1ps, s2ps, first, last,
                         np_=P, eoff_half=None, acc_tag="acc1", wide=False):
                """Gather + theta + max over neighbors + sum over S + fc add.
                If eoff_half is not None: two-half gather/theta with element
                offsets (conv4); out_ap may be None (acc kept for collective)."""
                abufs = 1 if acc_tag == "acc4" else 2
                acc = st.tile([np_, supw], F32, tag=acc_tag, name=acc_tag, bufs=abufs)
                halves = [(0, supw)] if eoff_half is None else eoff_half
                for j in range(nslots):
                    for hi, (e0, en) in enumerate(halves):
                        g = TS((np_, en), tag="cvg")
                        nc.gpsimd.indirect_dma_start(
                            out=g[:], out_offset=None, in_=sup_tab[:],
                            in_offset=bass.IndirectOffsetOnAxis(
                                ap=nidx_tile[:, col0 + j:col0 + j + 1], axis=0),
                            element_offset=e0)
                        tp = P1((np_, en), tag="wide")
                        for c0, cn in _nchunks(en):
                            nc.tensor.matmul(out=tp[:, c0:c0 + cn], lhsT=ndnT_ap(j),
                                             rhs=sdnt[:, e0 + c0:e0 + c0 + cn],
                                             start=True, stop=True)
                        th = TS((np_, en), tag="big8")
                        nc.scalar.activation(out=th[:], in_=tp[:], func=AF.Relu,
                                             scale=rinv_ap(j))
                        if j == 0:
                            nc.vector.tensor_tensor(out=acc[:, e0:e0 + en], in0=th[:],
                                                    in1=g[:], op=OP.mult)
                        else:
                            nc.vector.tensor_tensor(out=th[:], in0=th[:], in1=g[:], op=OP.mult)
                            nc.vector.tensor_tensor(out=acc[:, e0:e0 + en],
                                                    in0=acc[:, e0:e0 + en], in1=th[:], op=OP.max)
                if out_ap is None:
                    return acc
                nc.vector.tensor_add(out=out_ap, in0=acc[:, 0:oc], in1=acc[:, oc:2 * oc])
                for s in range(2, S):
                    nc.vector.tensor_add(out=out_ap, in0=out_ap, in1=acc[:, s * oc:(s + 1) * oc])
                nc.vector.tensor_add(out=out_ap, in0=out_ap, in1=fc_ap)
                # stats
                sqx = TS((np_, oc), tag="cvsq")
                nc.scalar.activation(out=sqx[:], in_=out_ap, func=AF.Square)
                nc.tensor.matmul(out=s1ps[:], lhsT=onesc[:np_, :], rhs=out_ap,
                                 start=first, stop=last)
                nc.tensor.matmul(out=s2ps[:], lhsT=onesc[:np_, :], rhs=sqx[:],
                                 start=first, stop=last)
                return None

            fm1p_t = [T((P, 128), tag=f"rowt_{t}") for t in range(8)]
            s1_1 = PA((1, 128), tag="s1acc")
            s2_1 = PA((1, 128), tag="s2acc")
            ag1i = DT((VL, 128), tag="ag1i")
            for t in range(8):
                build_ndn(t, nidx_all[:, t * 16:(t + 1) * 16], 1, verts,
                          qverts[t * P:(t + 1) * P, :],
                          lambda j, t=t: ndnT(t, j),
                          rinv[:, t * 10:(t + 1) * 10])
                conv_act(t, 10, t * 16 + 1, nidx_all, f1sup, SUP1, sdn1t,
                         lambda j, t=t: rinv[:, t * 10 + j:t * 10 + j + 1],
                         lambda j, t=t: ndnT(t, j),
                         128, fc1[:, t * P:(t + 1) * P], fm1p_t[t][:],
                         s1_1, s2_1, t == 0, t == 7, acc_tag="cvacc")
                nc.sync.dma_start(out=ag1i[t * P:(t + 1) * P, :], in_=fm1p_t[t][:])

            st1 = T((1, 256), tag="stx")
            nc.scalar.copy(out=st1[:, 0:128], in_=s1_1[:])
            nc.scalar.copy(out=st1[:, 128:256], in_=s2_1[:])
            ar1i = DT((1, 256), tag="ar1i")
            ar1o = DT((1, 256), tag="ar1o")
            nc.sync.dma_start(out=ar1i[:], in_=st1[:])
            nc.gpsimd.collective_compute("AllReduce", OP.add, replica_groups=ALL8,
                                         ins=[ar1i[:]], outs=[ar1o[:]])
            ag1o = DT((V, 128), tag="ag1o")
            nc.gpsimd.collective_compute("AllGather", OP.bypass, replica_groups=PAIRS,
                                         ins=[ag1i[:]], outs=[ag1o[:]])

            st1g = T((1, 256), tag="stxg")
            nc.sync.dma_start(out=st1g[:], in_=ar1o[:])
            sc1r, bi1r = bn_rows(st1g, 128, B * V, bngt[:, 64:192], bnbt[:, 64:192], "bn1")
            bs1, bb1 = bcast_rows(sc1r, bi1r, 128, "bn1")

            fm1T_loc = T((P, VL), tag="fmTloc")
            for t in range(8):
                nc.vector.tensor_tensor(out=fm1p_t[t][:], in0=fm1p_t[t][:], in1=bs1[:], op=OP.mult)
                nc.vector.tensor_tensor(out=fm1p_t[t][:], in0=fm1p_t[t][:], in1=bb1[:], op=OP.add)
                nc.scalar.activation(out=fm1p_t[t][:], in_=fm1p_t[t][:], func=AF.Relu)
                tp = PT((P, P), tag="sm")
                nc.tensor.transpose(out=tp[:], in_=fm1p_t[t][:], identity=identf[:])
                nc.scalar.copy(out=fm1T_loc[:, t * P:(t + 1) * P], in_=tp[:])
            nc.sync.dma_start(out=out_fuse[128:256, :], in_=fm1T_loc[:])

            for m in range(16):
                ft = TS((P, 128), tag="agt")
                nc.sync.dma_start(out=ft[:], in_=ag1o[m * P:(m + 1) * P, :])
                nc.vector.tensor_tensor(out=ft[:], in0=ft[:], in1=bs1[:], op=OP.mult)
                nc.vector.tensor_tensor(out=ft[:], in0=ft[:], in1=bb1[:], op=OP.add)
                nc.scalar.activation(out=ft[:], in_=ft[:], func=AF.Relu)
                nc.sync.dma_start(out=fm1_tab[m * P:(m + 1) * P, :], in_=ft[:])

            # ================= pool1 =================
            # pooled_nidx: nidx cols 1..4 at local rows 4*j
            p1n = [T((P, 4), U32, tag=f"p1n_{jt}") for jt in range(2)]
            for jt in range(2):
                for tp_ in range(4):
                    t = 4 * jt + tp_
                    nc.sync.dma_start(
                        out=p1n[jt][tp_ * 32:(tp_ + 1) * 32, 0:4],
                        in_=nidx_all[0::4, t * 16 + 1:t * 16 + 5])
            fmp1_t = [T((P, 128), tag=f"fmp1_{jt}") for jt in range(2)]
            agp1i = DT((V1L, 128), tag="agp1i")
            for jt in range(2):
                for i in range(4):
                    pg = TS((P, 128), tag="gu")
                    nc.gpsimd.indirect_dma_start(
                        out=pg[:], out_offset=None, in_=fm1_tab[:],
                        in_offset=bass.IndirectOffsetOnAxis(ap=p1n[jt][:, i:i + 1], axis=0))
                    if i == 0:
                        nc.vector.tensor_copy(out=fmp1_t[jt][:], in_=pg[:])
                    else:
                        nc.vector.tensor_tensor(out=fmp1_t[jt][:], in0=fmp1_t[jt][:],
                                                in1=pg[:], op=OP.max)
                nc.sync.dma_start(out=agp1i[jt * P:(jt + 1) * P, :], in_=fmp1_t[jt][:])
            agp1o = DT((V1, 128), tag="agp1o")
            nc.gpsimd.collective_compute("AllGather", OP.bypass, replica_groups=PAIRS,
                                         ins=[agp1i[:]], outs=[agp1o[:]])
            fmp1T_loc = T((P, V1L), tag="fmTlocB")
            for jt in range(2):
                tp = PT((P, P), tag="sm")
                nc.tensor.transpose(out=tp[:], in_=fmp1_t[jt][:], identity=identf[:])
                nc.scalar.copy(out=fmp1T_loc[:, jt * P:(jt + 1) * P], in_=tp[:])
            fmp1T_full = T((P, V1), tag="fmTfullB")
            for m in range(4):
                ft = TS((P, 128), tag="agt")
                nc.sync.dma_start(out=ft[:], in_=agp1o[m * P:(m + 1) * P, :])
                tp = PT((P, P), tag="sm")
                nc.tensor.transpose(out=tp[:], in_=ft[:], identity=identf[:])
                nc.scalar.copy(out=fmp1T_full[:, m * P:(m + 1) * P], in_=tp[:])

            # ================= KNN2 (local 256 queries over 512) ============
            nidx1 = T((P, 2 * 16), U32, tag="nidx1")
            vT2_1 = vT[:, 0::4]
            nbb1 = nbb[:, 0::4]
            for tt in range(2):
                sc = score_rows(qv1Tt[:, tt * P:(tt + 1) * P], vT2_1, nbb1, V1, "sc2")
                topk16(sc[:], nidx1[:, tt * 16:(tt + 1) * 16])
            nidx1x4 = T((P, 2 * 16), U32, tag="nidx1x4")
            nc.vector.tensor_scalar(out=nidx1x4[:], in0=nidx1[:], scalar1=2, scalar2=None,
                                    op0=OP.logical_shift_left)

            rinv2 = T((P, 2 * 10), tag="rinv2")

            def ndn2T(tt, j):
                return NDNB(f"s1_{tt % 2}")[:, j * P:(j + 1) * P]

            # ================= conv2 =================
            w2t = WSM()
            nc.sync.dma_start(out=w2t[:], in_=w2[:])
            b2t = BIA()
            nc.sync.dma_start(out=b2t[:, 0:256 + SUP2], in_=b2r[:])
            sdn2t = SDN()
            nc.sync.dma_start(out=sdn2t[:, 0:SUP2], in_=sdn2[:])

            for m in range(4):
                fo = P1((P, SUP2), tag="wide")
                for c0, cn in _nchunks(SUP2):
                    nc.tensor.matmul(out=fo[:, c0:c0 + cn],
                                     lhsT=fmp1T_full[:, m * P:(m + 1) * P],
                                     rhs=w2t[:, 256 + c0:256 + c0 + cn], start=True, stop=False)
                    nc.tensor.matmul(out=fo[:, c0:c0 + cn], lhsT=ones1[:],
                                     rhs=b2t[:, 256 + c0:256 + c0 + cn], start=False, stop=True)
                fos = TS((P, SUP2), tag="big8")
                nc.scalar.copy(out=fos[:], in_=fo[:])
                nc.sync.dma_start(out=f2sup[m * P:(m + 1) * P, :], in_=fos[:])
            fc2 = T((P, 2 * 256), tag="fcbuf")
            for tt in range(2):
                fp = PT((P, 256), tag="sm")
                nc.tensor.matmul(out=fp[:], lhsT=fmp1T_loc[:, tt * P:(tt + 1) * P],
                                 rhs=w2t[:, 0:256], start=True, stop=False)
                nc.tensor.matmul(out=fp[:], lhsT=ones1[:], rhs=b2t[:, 0:256],
                                 start=False, stop=True)
                nc.scalar.copy(out=fc2[:, tt * 256:(tt + 1) * 256], in_=fp[:])

            fm2p_t = [T((P, 256), tag=f"rowt2_{tt}") for tt in range(2)]
            s1_2 = PA((1, 256), tag="s1acc")
            s2_2 = PA((1, 256), tag="s2acc")
            ag2i = DT((V1L, 256), tag="ag2i")
            for tt in range(2):
                build_ndn(tt, nidx1x4[:, tt * 16:(tt + 1) * 16], 1, verts,
                          qv1[tt * P:(tt + 1) * P, :],
                          lambda j, tt=tt: ndn2T(tt, j),
                          rinv2[:, tt * 10:(tt + 1) * 10])
                conv_act(tt, 10, tt * 16 + 1, nidx1, f2sup, SUP2, sdn2t,
                         lambda j, tt=tt: rinv2[:, tt * 10 + j:tt * 10 + j + 1],
                         lambda j, tt=tt: ndn2T(tt, j),
                         256, fc2[:, tt * 256:(tt + 1) * 256], fm2p_t[tt][:],
                         s1_2, s2_2, tt == 0, tt == 1, acc_tag="cvacc", wide=True)
                nc.sync.dma_start(out=ag2i[tt * P:(tt + 1) * P, :], in_=fm2p_t[tt][:])

            st2 = T((1, 512), tag="stx")
            nc.scalar.copy(out=st2[:, 0:256], in_=s1_2[:])
            nc.scalar.copy(out=st2[:, 256:512], in_=s2_2[:])
            ar2i = DT((1, 512), tag="ar2i")
            ar2o = DT((1, 512), tag="ar2o")
            nc.sync.dma_start(out=ar2i[:], in_=st2[:])
            nc.gpsimd.collective_compute("AllReduce", OP.add, replica_groups=ALL8,
                                         ins=[ar2i[:]], outs=[ar2o[:]])
            ag2o = DT((V1, 256), tag="ag2o")
            nc.gpsimd.collective_compute("AllGather", OP.bypass, replica_groups=PAIRS,
                                         ins=[ag2i[:]], outs=[ag2o[:]])

            st2g = T((1, 512), tag="stxg")
            nc.sync.dma_start(out=st2g[:], in_=ar2o[:])
            sc2r, bi2r = bn_rows(st2g, 256, B * V1, bngt[:, 192:448], bnbt[:, 192:448], "bn2")
            bs2, bb2 = bcast_rows(sc2r, bi2r, 256, "bn2")

            # local fm2 (for conv3 pass B lhsT)
            fm2T_loc = [T((P, V1L), tag=("fmTlocB" if k == 0 else "fmTlocC")) for k in range(2)]
            for tt in range(2):
                nc.vector.tensor_tensor(out=fm2p_t[tt][:], in0=fm2p_t[tt][:], in1=bs2[:], op=OP.mult)
                nc.vector.tensor_tensor(out=fm2p_t[tt][:], in0=fm2p_t[tt][:], in1=bb2[:], op=OP.add)
                nc.scalar.activation(out=fm2p_t[tt][:], in_=fm2p_t[tt][:], func=AF.Relu)
                for k in range(2):
                    tp = PT((P, P), tag="sm")
                    nc.tensor.transpose(out=tp[:], in_=fm2p_t[tt][:, k * P:(k + 1) * P],
                                        identity=identf[:])
                    nc.scalar.copy(out=fm2T_loc[k][:, tt * P:(tt + 1) * P], in_=tp[:])
            fm2T_full = [T((P, V1), tag=("fmTfullA" if k == 0 else "fmTfullB")) for k in range(2)]
            for m in range(4):
                ft = TS((P, 256), tag="agt")
                nc.sync.dma_start(out=ft[:], in_=ag2o[m * P:(m + 1) * P, :])
                nc.vector.tensor_tensor(out=ft[:], in0=ft[:], in1=bs2[:], op=OP.mult)
                nc.vector.tensor_tensor(out=ft[:], in0=ft[:], in1=bb2[:], op=OP.add)
                nc.scalar.activation(out=ft[:], in_=ft[:], func=AF.Relu)
                nc.sync.dma_start(out=fm2_tab[m * P:(m + 1) * P, :], in_=ft[:])
                for k in range(2):
                    tp = PT((P, P), tag="sm")
                    nc.tensor.transpose(out=tp[:], in_=ft[:, k * P:(k + 1) * P],
                                        identity=identf[:])
                    nc.scalar.copy(out=fm2T_full[k][:, m * P:(m + 1) * P], in_=tp[:])

            # ================= conv3 =================
            b3t = BIA()
            nc.sync.dma_start(out=b3t[:, 0:256 + SUP3], in_=b3r[:])
            sdn3t = SDN()
            nc.sync.dma_start(out=sdn3t[:, 0:SUP3], in_=sdn3[:])

            for m in range(4):
                fo = P1((P, SUP3), tag="wide")
                for c0, cn in _nchunks(SUP3):
                    wa = TS((P, 512), tag="wAc")
                    nc.sync.dma_start(out=wa[:, 0:cn], in_=w3[0:128, 256 + c0:256 + c0 + cn])
                    wb = TS((P, 512), tag="wBc")
                    nc.sync.dma_start(out=wb[:, 0:cn], in_=w3[128:256, 256 + c0:256 + c0 + cn])
                    nc.tensor.matmul(out=fo[:, c0:c0 + cn],
                                     lhsT=fm2T_full[0][:, m * P:(m + 1) * P],
                                     rhs=wa[:, 0:cn], start=True, stop=False)
                    nc.tensor.matmul(out=fo[:, c0:c0 + cn],
                                     lhsT=fm2T_full[1][:, m * P:(m + 1) * P],
                                     rhs=wb[:, 0:cn], start=False, stop=False)
                    nc.tensor.matmul(out=fo[:, c0:c0 + cn], lhsT=ones1[:],
                                     rhs=b3t[:, 256 + c0:256 + c0 + cn], start=False, stop=True)
                fos = TS((P, SUP3), tag="big8")
                nc.scalar.copy(out=fos[:], in_=fo[:])
                nc.sync.dma_start(out=f3sup[m * P:(m + 1) * P, :], in_=fos[:])
            fc3 = T((P, 2 * 256), tag="fcbuf")
            for tt in range(2):
                wa = TS((P, 512), tag="wAc")
                nc.sync.dma_start(out=wa[:, 0:256], in_=w3[0:128, 0:256])
                wb = TS((P, 512), tag="wBc")
                nc.sync.dma_start(out=wb[:, 0:256], in_=w3[128:256, 0:256])
                fp = PT((P, 256), tag="sm")
                nc.tensor.matmul(out=fp[:], lhsT=fm2T_loc[0][:, tt * P:(tt + 1) * P],
                                 rhs=wa[:, 0:256], start=True, stop=False)
                nc.tensor.matmul(out=fp[:], lhsT=fm2T_loc[1][:, tt * P:(tt + 1) * P],
                                 rhs=wb[:, 0:256], start=False, stop=False)
                nc.tensor.matmul(out=fp[:], lhsT=ones1[:], rhs=b3t[:, 0:256],
                                 start=False, stop=True)
                nc.scalar.copy(out=fc3[:, tt * 256:(tt + 1) * 256], in_=fp[:])

            fm3p_t = [T((P, 256), tag=f"rowt2_{tt}") for tt in range(2)]
            s1_3 = PA((1, 256), tag="s1acc")
            s2_3 = PA((1, 256), tag="s2acc")
            ag3i = DT((V1L, 256), tag="ag3i")
            for tt in range(2):
                build_ndn(tt, nidx1x4[:, tt * 16:(tt + 1) * 16], 1, verts,
                          qv1[tt * P:(tt + 1) * P, :],
                          lambda j, tt=tt: ndn2T(tt, j),
                          rinv2[:, tt * 10:(tt + 1) * 10])
                conv_act(tt, 10, tt * 16 + 1, nidx1, f3sup, SUP3, sdn3t,
                         lambda j, tt=tt: rinv2[:, tt * 10 + j:tt * 10 + j + 1],
                         lambda j, tt=tt: ndn2T(tt, j),
                         256, fc3[:, tt * 256:(tt + 1) * 256], fm3p_t[tt][:],
                         s1_3, s2_3, tt == 0, tt == 1, acc_tag="cvacc", wide=True)
                nc.sync.dma_start(out=ag3i[tt * P:(tt + 1) * P, :], in_=fm3p_t[tt][:])

            st3 = T((1, 512), tag="stx")
            nc.scalar.copy(out=st3[:, 0:256], in_=s1_3[:])
            nc.scalar.copy(out=st3[:, 256:512], in_=s2_3[:])
            ar3i = DT((1, 512), tag="ar3i")
            ar3o = DT((1, 512), tag="ar3o")
            nc.sync.dma_start(out=ar3i[:], in_=st3[:])
            nc.gpsimd.collective_compute("AllReduce", OP.add, replica_groups=ALL8,
                                         ins=[ar3i[:]], outs=[ar3o[:]])
            ag3o = DT((V1, 256), tag="ag3o")
            nc.gpsimd.collective_compute("AllGather", OP.bypass, replica_groups=PAIRS,
                                         ins=[ag3i[:]], outs=[ag3o[:]])

            st3g = T((1, 512), tag="stxg")
            nc.sync.dma_start(out=st3g[:], in_=ar3o[:])
            sc3r, bi3r = bn_rows(st3g, 256, B * V1, bngt[:, 448:704], bnbt[:, 448:704], "bn3")
            bs3, bb3 = bcast_rows(sc3r, bi3r, 256, "bn3")

            for m in range(4):
                ft = TS((P, 256), tag="agt")
                nc.sync.dma_start(out=ft[:], in_=ag3o[m * P:(m + 1) * P, :])
                nc.vector.tensor_tensor(out=ft[:], in0=ft[:], in1=bs3[:], op=OP.mult)
                nc.vector.tensor_tensor(out=ft[:], in0=ft[:], in1=bb3[:], op=OP.add)
                nc.scalar.activation(out=ft[:], in_=ft[:], func=AF.Relu)
                nc.sync.dma_start(out=fm3_tab[m * P:(m + 1) * P, :], in_=ft[:])

            # ================= pool2 + fmp2 =================
            p2n = T((V2L, 4), U32, tag="p2n")
            nc.sync.dma_start(out=p2n[0:32, 0:4], in_=nidx1[0::4, 1:5])
            nc.sync.dma_start(out=p2n[32:64, 0:4], in_=nidx1[0::4, 17:21])
            fmp2_loc = T((V2L, 256), tag="fmp2_loc")
            for i in range(4):
                pg = TS((V2L, 256), tag="gu")
                nc.gpsimd.indirect_dma_start(
                    out=pg[:], out_offset=None, in_=fm3_tab[:],
                    in_offset=bass.IndirectOffsetOnAxis(ap=p2n[:, i:i + 1], axis=0))
                if i == 0:
                    nc.vector.tensor_copy(out=fmp2_loc[:], in_=pg[:])
                else:
                    nc.vector.tensor_tensor(out=fmp2_loc[:], in0=fmp2_loc[:], in1=pg[:], op=OP.max)
            agp2i = DT((V2L, 256), tag="agp2i")
            nc.sync.dma_start(out=agp2i[:], in_=fmp2_loc[:])
            agp2o = DT((V2, 256), tag="agp2o")
            nc.gpsimd.collective_compute("AllGather", OP.bypass, replica_groups=PAIRS,
                                         ins=[agp2i[:]], outs=[agp2o[:]])
            fmp2f = T((P, 256), tag="fmp2f")
            nc.sync.dma_start(out=fmp2f[:], in_=agp2o[:])
            fmp2T = [T((P, P), tag=f"fmp2T{k}") for k in range(2)]
            for k in range(2):
                tp = PT((P, P), tag="sm")
                nc.tensor.transpose(out=tp[:], in_=fmp2f[:, k * P:(k + 1) * P], identity=identf[:])
                nc.scalar.copy(out=fmp2T[k][:], in_=tp[:])

            # ================= KNN3 (all 128 over 128) =================
            nidx2 = T((P, 16), U32, tag="nidx2")
            qv2 = TS((P, 3), tag="qc")
            nc.sync.dma_start(out=qv2[:], in_=verts[0::16, :])
            qv2T_ps = PT((3, P), tag="sm")
            nc.tensor.transpose(out=qv2T_ps[:], in_=qv2[:], identity=identf[:])
            qv2T = T((3, P), tag="qv2T")
            nc.scalar.copy(out=qv2T[:], in_=qv2T_ps[:])
            sc = score_rows(qv2T[:], vT[:, 0::16], nbb[:, 0::16], V2, "sc3")
            topk16(sc[:], nidx2[:])
            # blend even/odd neighbor slots by h: core h takes slots {1+h,3+h,..}
            hb_ps = PT((P, 1), tag="sm")
            nc.tensor.matmul(out=hb_ps[:], lhsT=ones1[:], rhs=hmt[:], start=True, stop=True)
            hb = T((P, 1), tag="hb")
            nc.scalar.copy(out=hb[:], in_=hb_ps[:])
            ihb = T((P, 1), tag="ihb")
            nc.vector.tensor_scalar(out=ihb[:], in0=hb[:], scalar1=-1.0, scalar2=1.0,
                                    op0=OP.mult, op1=OP.add)
            n2f = T((P, 16), tag="n2f")
            nc.vector.tensor_copy(out=n2f[:], in_=nidx2[:])
            seln_f = T((P, 5), tag="seln_f")
            for j in range(5):
                a = TS((P, 1), tag="bl_a")
                nc.vector.tensor_tensor(out=a[:], in0=n2f[:, 1 + 2 * j:2 + 2 * j],
                                        in1=ihb[:], op=OP.mult)
                bsel = TS((P, 1), tag="bl_b")
                nc.vector.tensor_tensor(out=bsel[:], in0=n2f[:, 2 + 2 * j:3 + 2 * j],
                                        in1=hb[:], op=OP.mult)
                nc.vector.tensor_tensor(out=seln_f[:, j:j + 1], in0=a[:], in1=bsel[:], op=OP.add)
            seln = T((P, 5), U32, tag="seln")
            nc.vector.tensor_copy(out=seln[:], in_=seln_f[:])
            selnx16 = T((P, 5), U32, tag="selnx16")
            nc.vector.tensor_scalar(out=selnx16[:], in0=seln[:], scalar1=4, scalar2=None,
                                    op0=OP.logical_shift_left)

            rinv3 = T((P, 5), tag="rinv3")

            def ndn3T(j):
                return NDNB("s1_0")[:, j * P:(j + 1) * P]

            build_ndn(0, selnx16, 0, verts, verts[0::16, :],
                      lambda j: ndn3T(j), rinv3[:], nslots=5)

            # ================= f_out4 =================
            b4t = BIA()
            nc.sync.dma_start(out=b4t[:], in_=b4r[:])
            sdn4t = SDN()
            nc.sync.dma_start(out=sdn4t[:], in_=sdn4[:])

            fc4 = T((P, 512), tag="fcbuf")
            for ch in range(4):
                fo = P1((P, 1024), tag="wide")
                base = ch * 1024
                for c0, cn in _nchunks(1024):
                    wa = TS((P, 512), tag="wAc")
                    nc.sync.dma_start(out=wa[:, 0:cn], in_=w4[0:128, base + c0:base + c0 + cn])
                    wb = TS((P, 512), tag="wBc")
                    nc.sync.dma_start(out=wb[:, 0:cn], in_=w4[128:256, base + c0:base + c0 + cn])
                    nc.tensor.matmul(out=fo[:, c0:c0 + cn], lhsT=fmp2T[0][:],
                                     rhs=wa[:, 0:cn], start=True, stop=False)
                    nc.tensor.matmul(out=fo[:, c0:c0 + cn], lhsT=fmp2T[1][:],
                                     rhs=wb[:, 0:cn], start=False, stop=False)
                    nc.tensor.matmul(out=fo[:, c0:c0 + cn], lhsT=ones1[:],
                                     rhs=b4t[:, base + c0:base + c0 + cn], start=False, stop=True)
                fos = TS((P, 1024), tag="big8")
                nc.scalar.copy(out=fos[:], in_=fo[:])
                if ch == 0:
                    nc.vector.tensor_copy(out=fc4[:], in_=fos[:, 0:512])
                    nc.sync.dma_start(out=f4sup[:, 0:512], in_=fos[:, 512:1024])
                else:
                    nc.sync.dma_start(out=f4sup[:, base - 512:base + 512], in_=fos[:])

            # ================= conv4 (slot-split 5/5, max-allreduce) =========
            acc4 = conv_act(0, 5, 0, seln, f4sup, SUP4, sdn4t,
                            lambda j: rinv3[:, j:j + 1],
                            lambda j: ndn3T(j),
                            512, None, None, None, None, True, True,
                            eoff_half=[(0, 1792), (1792, 1792)], acc_tag="acc4", wide=True)
            ar4i = DT((P, SUP4), tag="ar4i")
            nc.sync.dma_start(out=ar4i[:], in_=acc4[:])
            ar4o = DT((P, SUP4), tag="ar4o")
            nc.gpsimd.collective_compute("AllReduce", OP.max, replica_groups=PAIRS,
                                         ins=[ar4i[:]], outs=[ar4o[:]])
            acc4f = st.tile([P, SUP4], F32, tag="acc4", name="acc4f", bufs=1)
            nc.sync.dma_start(out=acc4f[:], in_=ar4o[:])
            fm4 = T((P, 512), tag="fm4")
            nc.vector.tensor_add(out=fm4[:], in0=acc4f[:, 0:512], in1=acc4f[:, 512:1024])
            for s in range(2, S):
                nc.vector.tensor_add(out=fm4[:], in0=fm4[:], in1=acc4f[:, s * 512:(s + 1) * 512])
            nc.vector.tensor_add(out=fm4[:], in0=fm4[:], in1=fc4[:])
            nc.sync.dma_start(out=fm4_tab[:], in_=fm4[:])

            # f_global: max over the 128 v2 rows -> (512,), broadcast to fuse rows
            for k in range(4):
                tp = PT((P, P), tag="sm")
                nc.tensor.transpose(out=tp[:], in_=fm4[:, k * P:(k + 1) * P], identity=identf[:])
                fmT = TS((P, P), tag="fm4T")
                nc.scalar.copy(out=fmT[:], in_=tp[:])
                fg = TS((P, 1), tag="fg")
                nc.vector.tensor_reduce(out=fg[:], in_=fmT[:], axis=AX, op=OP.max)
                fgb = TS((P, VL), tag="big8")
                nc.vector.tensor_scalar(out=fgb[:], in0=fm1T_loc[:], scalar1=0.0,
                                        scalar2=fg[:, 0:1], op0=OP.mult, op1=OP.add)
                nc.sync.dma_start(out=out_fuse[1280 + k * P:1280 + (k + 1) * P, :], in_=fgb[:])

            # ================= near1/near2 + upsample =================
            vT2_2 = vT[:, 0::16]
            nbb2 = nbb[:, 0::16]
            for t in range(8):
                sc1 = score_rows(qvTt[:, t * P:(t + 1) * P], vT2_1, nbb1, V1, "scn1")
                v8 = TS((P, 8), tag="v8a")
                i8a = T((P, 8), U32, tag="i8a")
                nc.vector.max(v8[:], sc1[:])
                nc.vector.max_index(i8a[:], v8[:], sc1[:])
                sc2 = score_rows(qvTt[:, t * P:(t + 1) * P], vT2_2, nbb2, V2, "scn2")
                v8b = TS((P, 8), tag="v8b")
                i8b = T((P, 8), U32, tag="i8b")
                nc.vector.max(v8b[:], sc2[:])
                nc.vector.max_index(i8b[:], v8b[:], sc2[:])

                for tab, idx_t, width, r0 in (
                    (fm2_tab, i8a, 256, 256),
                    (fm3_tab, i8a, 256, 512),
                    (fm4_tab, i8b, 512, 768),
                ):
                    gu = TS((P, width), tag="gu")
                    nc.gpsimd.indirect_dma_start(
                        out=gu[:], out_offset=None, in_=tab[:],
                        in_offset=bass.IndirectOffsetOnAxis(ap=idx_t[:, 0:1], axis=0))
                    for k in range(width // P):
                        tp = PT((P, P), tag="sm")
                        nc.tensor.transpose(out=tp[:], in_=gu[:, k * P:(k + 1) * P],
                                            identity=identf[:])
                        ot = TS((P, P), tag="otile")
                        nc.scalar.copy(out=ot[:], in_=tp[:])
                        nc.sync.dma_start(
                            out=out_fuse[r0 + k * P:r0 + (k + 1) * P, t * P:(t + 1) * P],
                            in_=ot[:])

    nc.compile()
    return nc


def _prep_in_maps(vertices, rgb_f, dir0, w1, b1, dir1, w2, b2, dir2, w3, b3, dir3,
                  w4, b4, dir4, rgb_w, rgb_b, rgb_bn_g, rgb_bn_b,
                  bn1_g, bn1_b, bn2_g, bn2_b, bn3_g, bn3_b):
    f32 = np.float32
    bf16 = ml_dtypes.bfloat16

    def norm_cols(d):
        n = np.linalg.norm(d.astype(np.float64), axis=0)
        return (d / np.maximum(n, 1e-12)).astype(f32)

    sdns = [norm_cols(d) for d in (dir0, dir1, dir2, dir3, dir4)]
    bng = np.concatenate([rgb_bn_g, bn1_g, bn2_g, bn3_g]).reshape(1, -1).astype(f32)
    bnb = np.concatenate([rgb_bn_b, bn1_b, bn2_b, bn3_b]).reshape(1, -1).astype(f32)
    ident = np.eye(P, dtype=f32)

    shared = dict(
        identin=ident,
        w1=np.ascontiguousarray(w1, f32), w2=np.ascontiguousarray(w2, f32),
        w3=np.ascontiguousarray(w3, f32), w4=np.ascontiguousarray(w4, f32),
        b1r=b1.reshape(1, -1).astype(f32), b2r=b2.reshape(1, -1).astype(f32),
        b3r=b3.reshape(1, -1).astype(f32), b4r=b4.reshape(1, -1).astype(f32),
        rgw=np.ascontiguousarray(rgb_w.T, f32), rgbbr=rgb_b.reshape(1, -1).astype(f32),
        sdn0=sdns[0], sdn1=sdns[1], sdn2=sdns[2], sdn3=sdns[3], sdn4=sdns[4],
        bngr=bng, bnbr=bnb,
    )
    in_maps = []
    for c in range(8):
        s, h = c // 2, c % 2
        vs = np.ascontiguousarray(vertices[s], f32)           # (V,3)
        vsT = np.ascontiguousarray(vs.T, f32)                 # (3,V)
        m = dict(shared)
        m["verts"] = vs
        m["vertsT"] = np.ascontiguousarray(2.0 * vsT, f32)
        m["qvT"] = np.ascontiguousarray(vsT[:, h * VL:(h + 1) * VL], f32)
        m["qverts"] = np.ascontiguousarray(vs[h * VL:(h + 1) * VL, :], f32)
        m["qv1"] = np.ascontiguousarray(vs[h * VL:(h + 1) * VL:4, :], f32)
        m["qv1T"] = np.ascontiguousarray(vs[h * VL:(h + 1) * VL:4, :].T, f32)
        m["rgbT"] = np.ascontiguousarray(rgb_f[s][:, h * VL:(h + 1) * VL], f32)
        m["hmask"] = np.array([[float(h)]], f32)
        in_maps.append(m)
    return in_maps


def kernel(**inputs):
    if "nc" not in _CACHE:
        _CACHE["nc"] = _build()
    nc = _CACHE["nc"]
    in_maps = _prep_in_maps(**inputs)
    res = run_bass_kernel_spmd(nc, in_maps, list(range(8))).results
    fuse = np.stack([
        np.concatenate([res[2 * s]["out_fuse"], res[2 * s + 1]["out_fuse"]], axis=1)
        for s in range(B)
    ]).astype(np.float32)
    feat = np.ascontiguousarray(fuse[:, :1280, :])
    return feat, fuse


def _timed_run(inputs, iters=8):
    """Steady-state per-call timing: device-resident inputs, compiled once.

    Mirrors bass2jax.run_bass_via_pjrt's multi-core path without donation so
    the same device buffers can be re-executed. Returns (best_ns, results).
    """
    import time
    import jax
    from concourse import bass2jax as b2j
    from concourse.bass2jax import (Mesh, PartitionSpec, shard_map,
                                    _bass_exec_p, partition_id_tensor,
                                    install_neuronx_cc_hook)

    if "nc" not in _CACHE:
        _CACHE["nc"] = _build()
    nc = _CACHE["nc"]
    in_maps = _prep_in_maps(**inputs)
    install_neuronx_cc_hook()

    import concourse.mybir as mb
    partition_name = nc.partition_id_tensor.name if nc.partition_id_tensor else None
    in_names, out_names, out_avals, zero_outs = [], [], [], []
    for alloc in nc.m.functions[0].allocations:
        if not isinstance(alloc, mb.MemoryLocationSet):
            continue
        name = alloc.memorylocations[0].name
        if alloc.kind == "ExternalInput":
            if name != partition_name:
                in_names.append(name)
        elif alloc.kind == "ExternalOutput":
            out_names.append(name)
            shape = tuple(alloc.tensor_shape)
            dtype = mb.dt.np(alloc.dtype)
            out_avals.append(jax.core.ShapedArray(shape, dtype))
            zero_outs.append(np.zeros(shape, dtype))
    n_params = len(in_names)
    all_names = list(in_names) + out_names + ([partition_name] if partition_name else [])

    def _body(*args):
        operands = list(args)
        if partition_name is not None:
            operands.append(partition_id_tensor())
        outs = _bass_exec_p.bind(
            *operands,
            out_avals=tuple(out_avals),
            in_names=tuple(all_names),
            out_names=tuple(out_names),
            lowering_input_output_aliases=(),
            sim_require_finite=True,
            sim_require_nnan=True,
            nc=nc,
        )
        return tuple(outs)

    devices = jax.devices()[:8]
    mesh = Mesh(np.asarray(devices), ("core",))
    nin = n_params + len(zero_outs)
    sharded = jax.jit(shard_map(
        _body, mesh=mesh, in_specs=(PartitionSpec("core"),) * nin,
        out_specs=(PartitionSpec("core"),) * len(out_names), check_rep=False))
    concat_in = [
        np.concatenate([np.asarray(in_maps[c][nm]) for c in range(8)], axis=0)
        for nm in in_names
    ] + [np.zeros((8 * z.shape[0], *z.shape[1:]), z.dtype) for z in zero_outs]
    from jax.sharding import NamedSharding
    sh = NamedSharding(mesh, PartitionSpec("core"))
    dev_in = [jax.device_put(a, sh) for a in concat_in]

    outs = sharded(*dev_in)
    jax.block_until_ready(outs)
    times = []
    for _ in range(iters):
        t0 = time.perf_counter()
        outs = sharded(*dev_in)
        jax.block_until_ready(outs)
        times.append(time.perf_counter() - t0)
    best_ns = min(times) * 1e9
    res = [{nm: np.asarray(outs[i]).reshape(8, *out_avals[i].shape)[c]
            for i, nm in enumerate(out_names)} for c in range(8)]
    return best_ns, res, times
